# revision 1
# baseline (speedup 1.0000x reference)
"""GQA attention layer (QKV proj + RoPE + softmax attention + out proj) on 8
Trainium2 NeuronCores.

Sharding: core c = (batch b, head-group g) with b = c // 4, g = c % 4.
Each core handles one batch and one GQA group (4 q heads + 1 kv head),
computes a w_o-column-sliced partial output (row-parallel), and the host sums
the 4 partials per batch (the "all-reduce" of the hinted TP scheme, done on
host since the final gather happens there anyway).
"""

import numpy as np
import ml_dtypes

B, S, HID = 2, 2048, 1024
NH, NKV, D = 16, 4, 64
SCALE = D ** -0.5
NCORES = 8
TT = 512          # token tile (projection N / q tile)
NTT = S // TT     # 4
KC = S // 128     # 16 k chunks

_BF16 = ml_dtypes.bfloat16

_nc_cache = None


def _build_bass():
    import concourse.bass as bass
    import concourse.mybir as mybir
    import concourse.tile as tile
    from concourse import bacc
    from concourse.masks import make_identity

    BF = mybir.dt.bfloat16
    F32 = mybir.dt.float32
    AF = mybir.ActivationFunctionType
    MULT = mybir.AluOpType.mult
    ADD = mybir.AluOpType.add

    nc = bacc.Bacc()
    hT = nc.dram_tensor("hT", (HID, S), BF, kind="ExternalInput")
    wqkT = nc.dram_tensor("wqkT", (HID, 384), BF, kind="ExternalInput")
    woT = nc.dram_tensor("woT", (256, HID), BF, kind="ExternalInput")
    cosd = nc.dram_tensor("cosd", (64, S), F32, kind="ExternalInput")
    sind = nc.dram_tensor("sind", (64, S), F32, kind="ExternalInput")
    out = nc.dram_tensor("out", (S, HID), F32, kind="ExternalOutput")

    with tile.TileContext(nc) as tc:
        with (
            tc.tile_pool(name="persist", bufs=1) as pp,
            tc.tile_pool(name="rope", bufs=4) as rp,
            tc.tile_pool(name="exps", bufs=8) as ep,
            tc.tile_pool(name="norm", bufs=6) as np_,
            tc.tile_pool(name="outsb", bufs=6) as op_,
        ):
            # ---- persistent SBUF tiles + input loads (chunked for DMA spread)
            h_sb = pp.tile([128, 8, S], BF, tag="h_sb")
            wqk_sb = pp.tile([128, 8, 384], BF, tag="wqk_sb")
            wo_sb = pp.tile([128, 2, HID], BF, tag="wo_sb")
            cos_sb = pp.tile([64, S], F32, tag="cos_sb")
            sin_sb = pp.tile([64, S], F32, tag="sin_sb")
            h_dram = hT.rearrange("(c p) s -> p c s", p=128)
            wqk_dram = wqkT.rearrange("(c p) r -> p c r", p=128)
            wo_dram = woT.rearrange("(c p) h -> p c h", p=128)
            for hc in range(8):
                nc.sync.dma_start(wqk_sb[:, hc, :], wqk_dram[:, hc, :])
            for hc in range(8):
                nc.sync.dma_start(h_sb[:, hc, 0:TT], h_dram[:, hc, 0:TT])
            # rope tables right after the first token wave (first rope
            # needs them at ~10us; behind all hT they'd land too late)
            nc.sync.dma_start(cos_sb[:, : S // 2], cosd[:, : S // 2])
            nc.sync.dma_start(cos_sb[:, S // 2 :], cosd[:, S // 2 :])
            nc.sync.dma_start(sin_sb[:, : S // 2], sind[:, : S // 2])
            nc.sync.dma_start(sin_sb[:, S // 2 :], sind[:, S // 2 :])
            for tt in range(1, NTT):
                for hc in range(8):
                    tts_ = bass.ts(tt, TT)
                    nc.sync.dma_start(h_sb[:, hc, tts_], h_dram[:, hc, tts_])
            for oc in range(2):
                nc.sync.dma_start(wo_sb[:, oc, :], wo_dram[:, oc, :])

            ident = pp.tile([64, 64], BF, tag="ident")
            make_identity(nc, ident[:])
            ones64 = pp.tile([1, 64], F32, tag="ones64")
            nc.any.memset(ones64[:], 1.0)
            # preload the exp table set while input DMAs stream
            warm = pp.tile([1, 8], F32, tag="warm")
            nc.any.memset(warm[:], 0.0)
            nc.scalar.activation(warm[:], warm[:], AF.Exp)

            # roped q, 2 heads per tile (head 2p at rows 0:64, 2p+1 at 64:128)
            qrot = [pp.tile([128, S], BF, tag=f"qrot{p}", name=f"qrot{p}") for p in range(2)]
            # roped k duplicated on both partition halves
            k2 = pp.tile([128, S], BF, tag="k2")
            vT = pp.tile([64, S], BF, tag="vT")
            # V with ones column for fused softmax denominator
            vaug = pp.tile([128, KC, 65], BF, tag="vaug")
            nc.any.memset(vaug[:], 1.0)
            # normalized attention output (o-chunk tiles), [o, t] layout
            anorm = [pp.tile([128, S], BF, tag=f"anorm{o}", name=f"anorm{o}") for o in range(2)]

            def rope64(ps_blk, dests, tts, tmp_tag):
                """RoPE one 64-row head block [64, TT] (psum) -> dest slices."""
                t1 = rp.tile([64, TT], F32, tag=f"t1{tmp_tag}")
                rt = rp.tile([64, TT], F32, tag=f"rt{tmp_tag}")
                nc.vector.tensor_tensor(t1[:], ps_blk[0:64, :], cos_sb[:, tts], MULT)
                nc.vector.tensor_tensor(
                    rt[0:32, :], ps_blk[32:64, :], sin_sb[0:32, tts], MULT
                )
                nc.vector.tensor_tensor(
                    rt[32:64, :], ps_blk[0:32, :], sin_sb[32:64, tts], MULT
                )
                nc.vector.tensor_tensor(dests[0], t1[:], rt[:], ADD)
                for dest in dests[1:]:
                    # duplicate halves via the idle gpsimd engine
                    nc.gpsimd.tensor_copy(dest, dests[0])

            # ---- single PSUM layout: proj/misc 2 banks, scores 4, acc 2 ----
            F32R = mybir.dt.float32r
            with (
                tc.tile_pool(name="psP", bufs=2, space="PSUM") as psP,
                tc.tile_pool(name="psS", bufs=2, space="PSUM") as psS,
                tc.tile_pool(name="psACC", bufs=1, space="PSUM") as psACC,
            ):

                def proj_chunk(rc, only_tt=None):
                    for tt in range(NTT):
                        if only_tt is not None and tt != only_tt:
                            continue
                        tts = bass.ts(tt, TT)
                        ps = psP.tile([128, TT], F32, tag="proj",
                                      name=f"proj{rc}_{tt}")
                        for hc in range(8):
                            nc.tensor.matmul(
                                ps[:],
                                wqk_sb[:, hc, bass.ts(rc, 128)],
                                h_sb[:, hc, tts],
                                start=(hc == 0),
                                stop=(hc == 7),
                            )
                        if rc == 2:
                            nc.vector.tensor_copy(vT[:, tts], ps[64:128, :])
                            # k rows 0:64 -> rope -> both halves of k2
                            rope64(
                                ps[0:64, :],
                                [k2[0:64, tts], k2[64:128, tts]],
                                tts,
                                "k",
                            )
                        else:
                            rope64(ps[0:64, :], [qrot[rc][0:64, tts]], tts, "qa")
                            rope64(ps[64:128, :], [qrot[rc][64:128, tts]], tts, "qb")

                def attention_qt(pair, qt):
                        qts = bass.ts(qt, TT)
                        pacc = psACC.tile([65, 2 * TT], F32, tag="att",
                                          name=f"att{pair}_{qt}")
                        for c in range(KC):
                            cs = bass.ts(c, 128)
                            # both heads' score tiles side by side -> one exp
                            sc2 = psS.tile([128, 2 * TT], F32, tag="sc",
                                           name=f"sc{pair}_{qt}_{c}")
                            nc.tensor.matmul(
                                sc2[:, 0:TT],
                                k2[0:64, cs],
                                qrot[pair][0:64, qts],
                                start=True,
                                stop=True,
                                tile_position=(0, 0),
                            )
                            nc.tensor.matmul(
                                sc2[:, TT : 2 * TT],
                                k2[64:128, cs],
                                qrot[pair][64:128, qts],
                                start=True,
                                stop=True,
                                tile_position=(64, 0),
                            )
                            ex = ep.tile([128, 2 * TT], BF, tag="exp")
                            nc.scalar.activation(ex[:], sc2[:], AF.Exp)
                            for i in range(2):
                                nc.tensor.matmul(
                                    pacc[:, bass.ts(i, TT)],
                                    vaug[:, c, :],
                                    ex[:, bass.ts(i, TT)],
                                    start=(c == 0),
                                    stop=(c == KC - 1),
                                )
                        # normalize: attn[0:64] / attn[64] per head
                        for i in range(2):
                            its = bass.ts(i, TT)
                            den = np_.tile([1, TT], F32, tag="den")
                            nc.vector.tensor_copy(den[:], pacc[64:65, its])
                            rec = np_.tile([1, TT], F32, tag="rec")
                            nc.vector.reciprocal_approx_fast(rec[:], den[:])
                            bc = psP.tile([64, TT], F32, tag="proj",
                                          name=f"bc{pair}_{qt}_{i}")
                            nc.tensor.matmul(
                                bc[:], ones64[:], rec[:], start=True, stop=True
                            )
                            bcs = np_.tile([64, TT], F32, tag="bcs")
                            nc.vector.tensor_copy(bcs[:], bc[:])
                            nc.vector.tensor_tensor(
                                anorm[pair][bass.ts(i, 64), qts],
                                pacc[0:64, its],
                                bcs[:],
                                MULT,
                            )

                def transp_tt(tt_):
                    # V^T -> V transpose (PE) into vaug as soon as vT is ready
                    for c in range(4 * tt_, 4 * tt_ + 4):
                        pt = psP.tile([128, 64], BF, tag="proj", name=f"vt{c}")
                        nc.tensor.transpose(pt[:], vT[:, bass.ts(c, 128)],
                                            ident[:])
                        nc.vector.tensor_copy(vaug[:, c, 0:64], pt[:])

                for tt_ in range(NTT):
                    proj_chunk(2, only_tt=tt_)
                    transp_tt(tt_)
                proj_chunk(0, only_tt=0)
                # interleave remaining projections with pair-0 attention
                for qt in range(NTT):
                    if qt + 1 < NTT:
                        proj_chunk(0, only_tt=qt + 1)
                    proj_chunk(1, only_tt=qt)
                    attention_qt(0, qt)
                def outproj_qt(qt):
                    # token chunks whose anorm columns are complete after
                    # both pairs finished q-tile qt; the last q-tile borrows
                    # the scores pool, free once the final exp has drained
                    last = qt == NTT - 1
                    for tch in range(4 * qt, 4 * qt + 4):
                        tcs = bass.ts(tch, 128)
                        for ht in range(2):
                            hts = bass.ts(ht, TT)
                            # last q-tile: alternate between the two pools
                            # that are draining free (scores + proj)
                            pool, ptag = (
                                ((psS, "sc") if (tch + ht) % 2 else (psP, "proj"))
                                if last
                                else (psP, "proj")
                            )
                            po = pool.tile([128, TT], F32, tag=ptag,
                                           name=f"po{tch}_{ht}")
                            for oc in range(2):
                                nc.tensor.matmul(
                                    po[:],
                                    anorm[oc][:, tcs],
                                    wo_sb[:, oc, hts],
                                    start=(oc == 0),
                                    stop=(oc == 1),
                                )
                            ob = op_.tile([128, TT], F32, tag="ob")
                            nc.vector.tensor_copy(ob[:], po[:])
                            nc.sync.dma_start(out[tcs, hts], ob[:])

                for qt in range(NTT):
                    attention_qt(1, qt)
                    outproj_qt(qt)
    nc.finalize()
    return nc


def _get_nc():
    global _nc_cache
    if _nc_cache is None:
        _nc_cache = _build_bass()
    return _nc_cache


def _shard_inputs(hidden_states, cos, sin, w_qkv, w_o):
    """Build per-core input maps. Core c = (b = c // 4, g = c % 4)."""
    cosT = np.ascontiguousarray(cos.T.astype(np.float32))          # [64, S]
    sinT = sin.T.astype(np.float32)
    sinmod = np.concatenate([-sinT[0:32], sinT[32:64]], axis=0)    # sign folded
    sinmod = np.ascontiguousarray(sinmod)

    hT = [
        np.ascontiguousarray(hidden_states[b].T).astype(_BF16) for b in range(B)
    ]
    in_maps = []
    for c in range(NCORES):
        b, g = divmod(c, 4)
        q_rows = w_qkv[256 * g : 256 * g + 256] * SCALE
        k_rows = w_qkv[1024 + 64 * g : 1024 + 64 * g + 64]
        v_rows = w_qkv[1280 + 64 * g : 1280 + 64 * g + 64]
        wqk = np.concatenate([q_rows, k_rows, v_rows], axis=0)     # [384, 1024]
        wqkT = np.ascontiguousarray(wqk.T).astype(_BF16)           # [1024, 384]
        woT = np.ascontiguousarray(
            w_o[:, 256 * g : 256 * g + 256].T
        ).astype(_BF16)                                            # [256, 1024]
        in_maps.append(
            {
                "hT": hT[b],
                "wqkT": wqkT,
                "woT": woT,
                "cosd": cosT,
                "sind": sinmod,
            }
        )
    return in_maps


def _run(inputs, **spmd_kwargs):
    from concourse.bass_utils import run_bass_kernel_spmd

    nc = _get_nc()
    in_maps = _shard_inputs(**inputs)
    res = run_bass_kernel_spmd(
        nc, in_maps, core_ids=list(range(NCORES)), **spmd_kwargs
    )
    outs = []
    for b in range(B):
        acc = res.results[4 * b]["out"].astype(np.float32).copy()
        for g in range(1, 4):
            acc += res.results[4 * b + g]["out"]
        outs.append(acc)
    return np.stack(outs, axis=0), res


def kernel(**inputs):
    out, _ = _run(inputs)
    return out



# revision 4
# speedup vs baseline: 1.2663x; 1.2663x over previous
"""GQA attention layer on 8 Trainium2 NeuronCores — v2: filler-queue schedule.

Sharding: core c = (batch b = c//4, head-group g = c%4): 4 q heads + 1 kv head
per core, w_o row-parallel partial output, host sums the 4 partials per batch.

Schedule: attention (scores -> exp -> attnV) starts as soon as k-tile 0 is
roped; the remaining projection tiles, V transposes and outproj tiles are
injected as PE filler between attention chunk-pairs so the tensor engine
never stalls on the (pacing) Activation exp stream.
"""

import collections

import numpy as np
import ml_dtypes

B, S, HID = 2, 2048, 1024
NH, NKV, D = 16, 4, 64
SCALE = D ** -0.5
NCORES = 8
TT = 512          # token tile
NTT = S // TT     # 4
KC = S // 128     # 16 key chunks
NCP = KC // 2     # 8 chunk pairs

_BF16 = ml_dtypes.bfloat16

_nc_cache = None


def _build_bass():
    import concourse.bass as bass
    import concourse.mybir as mybir
    import concourse.tile as tile
    from concourse import bacc
    from concourse.masks import make_identity

    BF = mybir.dt.bfloat16
    F32 = mybir.dt.float32
    AF = mybir.ActivationFunctionType
    MULT = mybir.AluOpType.mult
    ADD = mybir.AluOpType.add

    nc = bacc.Bacc()
    hT = nc.dram_tensor("hT", (128, 8, S), BF, kind="ExternalInput")
    # rc-major: [p, rc, hc, m] so the kv slice (rc=2) can load first
    wqkT = nc.dram_tensor("wqkT", (128, 3, 8, 128), BF, kind="ExternalInput")
    woT = nc.dram_tensor("woT", (128, 2 * HID), BF, kind="ExternalInput")
    cosd = nc.dram_tensor("cosd", (128, S), BF, kind="ExternalInput")
    sind = nc.dram_tensor("sind", (128, S), BF, kind="ExternalInput")
    out = nc.dram_tensor("out", (S, HID), BF, kind="ExternalOutput")

    with tile.TileContext(nc) as tc:
        with (
            tc.tile_pool(name="persist", bufs=1) as pp,
            tc.tile_pool(name="pbfp", bufs=3) as pbfp,
            tc.tile_pool(name="rope", bufs=3) as rp,
            tc.tile_pool(name="exps", bufs=6) as ep,
            tc.tile_pool(name="norm", bufs=4) as np_,
            tc.tile_pool(name="outsb", bufs=4) as op_,
        ):
            # ---- persistent SBUF tiles + input loads. Order tuned so the
            # kv projection of token-tile 0 can start as early as possible:
            # kv weights -> h tile 0 -> rope tables tile 0 -> the rest.
            h_sb = pp.tile([128, 8, S], BF, tag="h_sb")
            wqk_sb = pp.tile([128, 3, 8, 128], BF, tag="wqk_sb")
            wo_sb = pp.tile([128, 2, HID], BF, tag="wo_sb")
            cos_sb = pp.tile([128, S], BF, tag="cos_sb")
            sin_sb = pp.tile([128, S], BF, tag="sin_sb")
            nc.sync.dma_start(wqk_sb[:, 2], wqkT[:, 2])
            nc.sync.dma_start(h_sb[:, :, 0:TT], hT[:, :, 0:TT])
            nc.sync.dma_start(cos_sb[:, 0:TT], cosd[:, 0:TT])
            nc.sync.dma_start(sin_sb[:, 0:TT], sind[:, 0:TT])
            nc.sync.dma_start(wqk_sb[:, 0:2], wqkT[:, 0:2])
            nc.sync.dma_start(h_sb[:, :, TT:2 * TT], hT[:, :, TT:2 * TT])
            nc.sync.dma_start(cos_sb[:, TT:], cosd[:, TT:])
            nc.sync.dma_start(sin_sb[:, TT:], sind[:, TT:])
            for tt in range(2, NTT):
                tts_ = bass.ts(tt, TT)
                nc.sync.dma_start(h_sb[:, :, tts_], hT[:, :, tts_])
            nc.sync.dma_start(wo_sb[:], woT[:])

            ident = pp.tile([64, 64], BF, tag="ident")
            make_identity(nc, ident[:])
            warm = pp.tile([1, 8], F32, tag="warm")
            nc.any.memset(warm[:], 0.0)
            nc.scalar.activation(warm[:], warm[:], AF.Exp)

            qrot = [pp.tile([128, S], BF, tag=f"qrot{p}", name=f"qrot{p}")
                    for p in range(2)]
            k2 = pp.tile([128, S], BF, tag="k2")
            vT = pp.tile([64, S], BF, tag="vT")
            vaug = pp.tile([128, KC, 65], BF, tag="vaug")
            nc.any.memset(vaug[:], 1.0)
            anorm = [pp.tile([128, S], BF, tag=f"anorm{o}", name=f"anorm{o}")
                     for o in range(2)]

            with (
                tc.tile_pool(name="psP", bufs=2, space="PSUM") as psP,
                tc.tile_pool(name="psS", bufs=2, space="PSUM") as psS,
                tc.tile_pool(name="psA", bufs=2, space="PSUM") as psA,
            ):

                def rope(pbf, dest, rows, tts, tag):
                    """pbf: bf16 SBUF copy of the proj tile. Pool builds the
                    32-block-swapped copy (SBUF->SBUF partition shift is Pool-
                    legal); DVE then runs same-partition bf16 2x-mode ops."""
                    sg = rp.tile([128, TT], BF, tag=f"sg{tag}")
                    for blk in range(rows // 32):
                        src = blk ^ 1
                        nc.gpsimd.tensor_copy(
                            sg[32 * blk: 32 * blk + 32, :],
                            pbf[32 * src: 32 * src + 32, :])
                    t1 = rp.tile([128, TT], BF, tag=f"t1{tag}")
                    nc.vector.tensor_tensor(
                        t1[0:rows, :], pbf[0:rows, :], cos_sb[0:rows, tts],
                        MULT)
                    rt = rp.tile([128, TT], BF, tag=f"rt{tag}")
                    nc.vector.tensor_tensor(
                        rt[0:rows, :], sg[0:rows, :], sin_sb[0:rows, tts],
                        MULT)
                    nc.vector.tensor_tensor(
                        dest, t1[0:rows, :], rt[0:rows, :], ADD)

                def proj_kv(tt):
                    tts = bass.ts(tt, TT)
                    ps = psP.tile([128, TT], F32, tag="proj",
                                  name=f"projkv_{tt}")
                    for hc in range(8):
                        nc.tensor.matmul(
                            ps[:], wqk_sb[:, 2, hc, :], h_sb[:, hc, tts],
                            start=(hc == 0), stop=(hc == 7))
                    kbf = pbfp.tile([128, TT], BF, tag="pbf", name=f"kbf{tt}")
                    nc.vector.tensor_copy(kbf[0:64, :], ps[0:64, :])
                    nc.vector.tensor_copy(vT[:, tts], ps[64:128, :])
                    rope(kbf, k2[0:64, tts], 64, tts, "k")
                    nc.gpsimd.tensor_copy(k2[64:128, tts], k2[0:64, tts])
                    for c in range(4 * tt, 4 * tt + 4):
                        pt = psP.tile([128, 64], BF, tag="proj", name=f"vt{c}")
                        nc.tensor.transpose(pt[:], vT[:, bass.ts(c, 128)],
                                            ident[:])
                        nc.vector.tensor_copy(vaug[:, c, 0:64], pt[:])

                def proj_q(rc, tt):
                    tts = bass.ts(tt, TT)
                    ps = psP.tile([128, TT], F32, tag="proj",
                                  name=f"projq{rc}_{tt}")
                    for hc in range(8):
                        nc.tensor.matmul(
                            ps[:], wqk_sb[:, rc, hc, :],
                            h_sb[:, hc, tts],
                            start=(hc == 0), stop=(hc == 7))
                    pbf = pbfp.tile([128, TT], BF, tag="pbf",
                                    name=f"qbf{rc}_{tt}")
                    nc.vector.tensor_copy(pbf[:], ps[:])
                    rope(pbf, qrot[rc][:, tts], 128, tts, "q")

                def outproj_tile(tch, ht, last=False):
                    tcs = bass.ts(tch, 128)
                    hts = bass.ts(ht, TT)
                    po = psP.tile([128, TT], F32, tag="proj",
                                  name=f"po{tch}_{ht}")
                    for oc in range(2):
                        nc.tensor.matmul(
                            po[:], anorm[oc][:, tcs], wo_sb[:, oc, hts],
                            start=(oc == 0), stop=(oc == 1))
                    ob = op_.tile([128, TT], BF, tag="ob")
                    if last:
                        # exp stream is drained by now; use the idle
                        # Activation engine for the tail copies
                        nc.scalar.copy(ob[:], po[:])
                    else:
                        nc.vector.tensor_copy(ob[:], po[:])
                    nc.sync.dma_start(out[tcs, hts], ob[:])

                # ---- filler queue: PE work injected between attention cpairs
                filler = collections.deque()

                def pump(n=1):
                    for _ in range(n):
                        if not filler:
                            return
                        filler.popleft()()

                # ---- attention for one head as a generator: one cpair per
                # step (scores x2, exp, previous attnV x2), norm at the end
                def attention_head(pair, h2, qt):
                    qts = bass.ts(qt, TT)
                    qrows = slice(64 * h2, 64 * h2 + 64)
                    pacc = psA.tile([65, TT], F32, tag="att",
                                    name=f"att{pair}_{h2}_{qt}")
                    pending = None
                    for cp in range(NCP):
                        sc = psS.tile([128, 2 * TT], F32, tag="sc",
                                      name=f"sc{pair}_{h2}_{qt}_{cp}")
                        for j in range(2):
                            c = 2 * cp + j
                            nc.tensor.matmul(
                                sc[:, bass.ts(j, TT)],
                                k2[qrows, bass.ts(c, 128)],
                                qrot[pair][qrows, qts],
                                start=True, stop=True,
                                tile_position=(64 * h2, 0))
                        ex = ep.tile([128, 2 * TT], BF, tag="exp")
                        nc.scalar.activation(ex[:], sc[:], AF.Exp)
                        pump(1)
                        if pending is not None:
                            pex, pcp = pending
                            for j in range(2):
                                c = 2 * pcp + j
                                nc.tensor.matmul(
                                    pacc[:], vaug[:, c, :],
                                    pex[:, bass.ts(j, TT)],
                                    start=(c == 0), stop=False)
                        pending = (ex, cp)
                        yield
                    pex, pcp = pending
                    for j in range(2):
                        c = 2 * pcp + j
                        nc.tensor.matmul(
                            pacc[:], vaug[:, c, :], pex[:, bass.ts(j, TT)],
                            start=False, stop=(c == KC - 1))
                    # reciprocal_approx_fast reading PSUM is broken on HW;
                    # stage the denominator row through SBUF first
                    den = np_.tile([1, TT], F32, tag="den")
                    nc.vector.tensor_copy(den[:], pacc[64:65, :])
                    rec = np_.tile([1, TT], F32, tag="rec")
                    nc.vector.reciprocal_approx_fast(rec[:], den[:])
                    bc = np_.tile([64, TT], F32, tag="bc")
                    nc.gpsimd.partition_broadcast(bc[:], rec[:])
                    nc.vector.tensor_tensor(
                        anorm[pair][qrows, qts], pacc[0:64, :], bc[:], MULT)

                # ---- master schedule: two attention-head generators run
                # alternately (wavefront over k tiles, 2 psA banks), all other
                # PE work is pumped as filler between cpairs
                proj_kv(0)
                proj_q(0, 0)
                filler.append(lambda: proj_kv(1))
                filler.append(lambda: proj_q(1, 0))
                filler.append(lambda: proj_kv(2))
                filler.append(lambda: proj_kv(3))
                for tt in range(1, NTT):
                    for rc in range(2):
                        filler.append(
                            lambda rc=rc, tt=tt: proj_q(rc, tt))

                heads = [(pair, h2, qt)
                         for qt in range(NTT)
                         for pair in range(2)
                         for h2 in range(2)]

                def head_done(i):
                    # after the 4th head of q-tile qt, its anorm columns are
                    # complete -> queue that q-tile's outproj as filler
                    if i % 4 == 3:
                        qt = heads[i][2]
                        last = i == len(heads) - 1
                        for tch in range(4 * qt, 4 * qt + 4):
                            for ht in range(2):
                                filler.append(
                                    lambda tch=tch, ht=ht, last=last:
                                    outproj_tile(tch, ht, last=last))

                nxt = 0

                def start_next():
                    nonlocal nxt
                    if nxt >= len(heads):
                        return None
                    g = attention_head(*heads[nxt])
                    nxt += 1
                    return (nxt - 1, g)

                slots = [start_next(), start_next()]
                while any(slots):
                    for si in range(2):
                        if slots[si] is None:
                            continue
                        i, g = slots[si]
                        try:
                            next(g)
                        except StopIteration:
                            head_done(i)
                            slots[si] = start_next()
                while filler:
                    pump(1)
    nc.finalize()
    return nc


def _get_nc():
    global _nc_cache
    if _nc_cache is None:
        _nc_cache = _build_bass()
    return _nc_cache


def _shard_inputs(hidden_states, cos, sin, w_qkv, w_o):
    """Build per-core input maps. Core c = (b = c // 4, g = c % 4)."""
    cosT = np.ascontiguousarray(cos.T.astype(np.float32))          # [64, S]
    sinT = sin.T.astype(np.float32)
    sinmod = np.concatenate([-sinT[0:32], sinT[32:64]], axis=0)    # sign folded
    cos2 = np.ascontiguousarray(np.concatenate([cosT, cosT], axis=0)
                                ).astype(_BF16)                    # [128, S]
    sin2 = np.ascontiguousarray(np.concatenate([sinmod, sinmod], axis=0)
                                ).astype(_BF16)

    # h packed [128, 8, S]: h_pk[p, hc, t] = hidden[b].T[hc*128 + p, t]
    hpk = []
    for b in range(B):
        ht = hidden_states[b].T.astype(_BF16)                      # [1024, S]
        hpk.append(np.ascontiguousarray(
            ht.reshape(8, 128, S).transpose(1, 0, 2)))             # [128,8,S]
    in_maps = []
    for c in range(NCORES):
        b, g = divmod(c, 4)
        q_rows = w_qkv[256 * g: 256 * g + 256] * SCALE
        k_rows = w_qkv[1024 + 64 * g: 1024 + 64 * g + 64]
        v_rows = w_qkv[1280 + 64 * g: 1280 + 64 * g + 64]
        wqk = np.concatenate([q_rows, k_rows, v_rows], axis=0)     # [384, 1024]
        wqkT = wqk.T.astype(_BF16)                                 # [1024, 384]
        # rc-major pack [128, 3, 8, 128]: [p, rc, hc, m] = wqkT[hc*128+p, rc*128+m]
        wqk_pk = np.ascontiguousarray(
            wqkT.reshape(8, 128, 3, 128).transpose(1, 2, 0, 3))
        woTf = w_o[:, 256 * g: 256 * g + 256].T.astype(_BF16)      # [256, 1024]
        wo_pk = np.ascontiguousarray(
            woTf.reshape(2, 128, HID).transpose(1, 0, 2).reshape(128, 2 * HID))
        in_maps.append(
            {
                "hT": hpk[b],
                "wqkT": wqk_pk,
                "woT": wo_pk,
                "cosd": cos2,
                "sind": sin2,
            }
        )
    return in_maps


def _run(inputs, **spmd_kwargs):
    from concourse.bass_utils import run_bass_kernel_spmd

    nc = _get_nc()
    in_maps = _shard_inputs(**inputs)
    res = run_bass_kernel_spmd(
        nc, in_maps, core_ids=list(range(NCORES)), **spmd_kwargs
    )
    outs = []
    for b in range(B):
        acc = res.results[4 * b]["out"].astype(np.float32)
        for g in range(1, 4):
            acc = acc + res.results[4 * b + g]["out"].astype(np.float32)
        outs.append(acc)
    return np.stack(outs, axis=0), res


def kernel(**inputs):
    out, _ = _run(inputs)
    return out


# revision 6
# speedup vs baseline: 1.3226x; 1.0445x over previous
"""GQA attention layer on 8 Trainium2 NeuronCores — v6.

v5 + fp8 hi/lo DoubleRow for both projections:
- QKV proj: host decomposes h and w_qkv into e4m3 hi + e5m2 lo; each K=256
  block is 3 DoubleRow matmuls (hi*hi + hi*lo + lo*hi) at half the PE cost
  of bf16, with ~bf16 accuracy (hi+lo carries ~14 mantissa bits).
- out proj: Pool computes anorm hi/lo from the bf16 normalize result;
  w_o decomposed on host. 3 DR matmuls replace 2 bf16 ones per tile.
Scaling: weights x16 into fp8 range, q rope tables fold SCALE/16, k tables
fold 1/16, V folds 1/16 at the vT copy, vaug ones-column = 1/32 so
anorm = 32*attn (fp8-friendly), w_o x16 -> output is 512x; host divides.
"""

import collections

import numpy as np
import ml_dtypes

B, S, HID = 2, 2048, 1024
NH, NKV, D = 16, 4, 64
SCALE = D ** -0.5
NCORES = 8
TT = 512          # token tile
NTT = S // TT     # 4
KC = S // 128     # 16 key chunks
NCP = KC // 2     # 8 chunk pairs
OUT_SCALE = 1.0 / 512.0

_BF16 = ml_dtypes.bfloat16
_E4 = ml_dtypes.float8_e4m3
_E5 = ml_dtypes.float8_e5m2

_nc_cache = None


def _build_bass():
    import concourse.bass as bass
    import concourse.mybir as mybir
    import concourse.tile as tile
    from concourse import bacc
    from concourse.masks import make_identity

    BF = mybir.dt.bfloat16
    F32 = mybir.dt.float32
    E4 = mybir.dt.float8e4
    E5 = mybir.dt.float8e5
    AF = mybir.ActivationFunctionType
    MULT = mybir.AluOpType.mult
    ADD = mybir.AluOpType.add
    SUB = mybir.AluOpType.subtract
    DR = mybir.MatmulPerfMode.DoubleRow

    nc = bacc.Bacc()
    # h split hi/lo, packed [p, j, i, t]: h feature 256j + 128i + p
    hhi = nc.dram_tensor("hhi", (128, 4, 2, S), E4, kind="ExternalInput")
    hlo = nc.dram_tensor("hlo", (128, 4, 2, S), E5, kind="ExternalInput")
    # wqk split hi/lo, packed [p, rc, j, i, m]
    whi = nc.dram_tensor("whi", (128, 3, 4, 2, 128), E4, kind="ExternalInput")
    wlo = nc.dram_tensor("wlo", (128, 3, 4, 2, 128), E5, kind="ExternalInput")
    wohi = nc.dram_tensor("wohi", (128, 2, HID), E4, kind="ExternalInput")
    wolo = nc.dram_tensor("wolo", (128, 2, HID), E5, kind="ExternalInput")
    cosd = nc.dram_tensor("cosd", (128, S), BF, kind="ExternalInput")  # q: *SCALE/16
    sind = nc.dram_tensor("sind", (128, S), BF, kind="ExternalInput")
    coskd = nc.dram_tensor("coskd", (64, S), BF, kind="ExternalInput")  # k: /16
    sinkd = nc.dram_tensor("sinkd", (64, S), BF, kind="ExternalInput")
    out = nc.dram_tensor("out", (S, HID), BF, kind="ExternalOutput")

    with tile.TileContext(nc) as tc:
        with (
            tc.tile_pool(name="persist", bufs=1) as pp,
            tc.tile_pool(name="pbfp", bufs=3) as pbfp,
            tc.tile_pool(name="rope", bufs=3) as rp,
            tc.tile_pool(name="exps", bufs=6) as ep,
            tc.tile_pool(name="norm", bufs=4) as np_,
            tc.tile_pool(name="outsb", bufs=4) as op_,
        ):
            # ---- persistent SBUF tiles + input loads, kv-first order
            hhi_sb = pp.tile([128, 4, 2, S], E4, tag="hhi_sb")
            hlo_sb = pp.tile([128, 4, 2, S], E5, tag="hlo_sb")
            whi_sb = pp.tile([128, 3, 4, 2, 128], E4, tag="whi_sb")
            wlo_sb = pp.tile([128, 3, 4, 2, 128], E5, tag="wlo_sb")
            wohi_sb = pp.tile([128, 2, HID], E4, tag="wohi_sb")
            wolo_sb = pp.tile([128, 2, HID], E5, tag="wolo_sb")
            cos_sb = pp.tile([128, S], BF, tag="cos_sb")
            sin_sb = pp.tile([128, S], BF, tag="sin_sb")
            cosk_sb = pp.tile([64, S], BF, tag="cosk_sb")
            sink_sb = pp.tile([64, S], BF, tag="sink_sb")

            def h_slice(t0, t1):
                for hd, hs in ((hhi, hhi_sb), (hlo, hlo_sb)):
                    nc.sync.dma_start(hs[:, :, :, t0:t1], hd[:, :, :, t0:t1])

            nc.sync.dma_start(whi_sb[:, 2], whi[:, 2])
            nc.sync.dma_start(wlo_sb[:, 2], wlo[:, 2])
            h_slice(0, TT)
            nc.sync.dma_start(cosk_sb[:, 0:TT], coskd[:, 0:TT])
            nc.sync.dma_start(sink_sb[:, 0:TT], sinkd[:, 0:TT])
            nc.sync.dma_start(cos_sb[:, 0:TT], cosd[:, 0:TT])
            nc.sync.dma_start(sin_sb[:, 0:TT], sind[:, 0:TT])
            nc.sync.dma_start(whi_sb[:, 0:2], whi[:, 0:2])
            nc.sync.dma_start(wlo_sb[:, 0:2], wlo[:, 0:2])
            h_slice(TT, 2 * TT)
            nc.sync.dma_start(cosk_sb[:, TT:], coskd[:, TT:])
            nc.sync.dma_start(sink_sb[:, TT:], sinkd[:, TT:])
            nc.sync.dma_start(cos_sb[:, TT:], cosd[:, TT:])
            nc.sync.dma_start(sin_sb[:, TT:], sind[:, TT:])
            h_slice(2 * TT, 3 * TT)
            h_slice(3 * TT, 4 * TT)
            nc.sync.dma_start(wohi_sb[:], wohi[:])
            nc.sync.dma_start(wolo_sb[:], wolo[:])

            ident = pp.tile([64, 64], BF, tag="ident")
            make_identity(nc, ident[:])
            warm = pp.tile([1, 8], F32, tag="warm")
            nc.any.memset(warm[:], 0.0)
            nc.scalar.activation(warm[:], warm[:], AF.Exp)

            qrot = [pp.tile([128, S], BF, tag=f"qrot{p}", name=f"qrot{p}")
                    for p in range(2)]
            k2 = pp.tile([128, S], BF, tag="k2")
            vT = pp.tile([64, S], BF, tag="vT")
            vaug = pp.tile([128, KC, 65], BF, tag="vaug")
            nc.any.memset(vaug[:], 1.0 / 32.0)
            # anorm = 32*attn: bf16 full + fp8 hi/lo for the DR outproj,
            # packed [p, oc(=pair), t]
            anorm = pp.tile([128, 2, S], BF, tag="anorm")
            ahi = pp.tile([128, 2, S], E4, tag="ahi")
            alo = pp.tile([128, 2, S], E5, tag="alo")

            with (
                tc.tile_pool(name="psP", bufs=2, space="PSUM") as psP,
                tc.tile_pool(name="psS", bufs=2, space="PSUM") as psS,
                tc.tile_pool(name="psA", bufs=2, space="PSUM") as psA,
            ):

                def rope(pbf, dest, rows, tts, ctab, stab, tag):
                    """Pool builds the 32-block-swapped copy; DVE runs
                    same-partition bf16 2x-mode multiply/add ops."""
                    sg = rp.tile([128, TT], BF, tag=f"sg{tag}")
                    for blk in range(rows // 32):
                        src = blk ^ 1
                        nc.gpsimd.tensor_copy(
                            sg[32 * blk: 32 * blk + 32, :],
                            pbf[32 * src: 32 * src + 32, :])
                    t1 = rp.tile([128, TT], BF, tag=f"t1{tag}")
                    nc.vector.tensor_tensor(
                        t1[0:rows, :], pbf[0:rows, :], ctab[0:rows, tts],
                        MULT)
                    rt = rp.tile([128, TT], BF, tag=f"rt{tag}")
                    nc.vector.tensor_tensor(
                        rt[0:rows, :], sg[0:rows, :], stab[0:rows, tts],
                        MULT)
                    nc.vector.tensor_tensor(
                        dest, t1[0:rows, :], rt[0:rows, :], ADD)

                def proj(rc, tt, name):
                    """hi/lo DoubleRow projection: 12 accumulating DR matmuls
                    (4 K=256 blocks x {hi*hi, hi*lo, lo*hi})."""
                    tts = bass.ts(tt, TT)
                    ps = psP.tile([128, TT], F32, tag="proj", name=name)
                    steps = []
                    for j in range(4):
                        steps.append((whi_sb[:, rc, j], hhi_sb[:, j, :, tts]))
                        steps.append((whi_sb[:, rc, j], hlo_sb[:, j, :, tts]))
                        steps.append((wlo_sb[:, rc, j], hhi_sb[:, j, :, tts]))
                    for si, (w, x) in enumerate(steps):
                        nc.tensor.matmul(
                            ps[:], w, x,
                            start=(si == 0), stop=(si == len(steps) - 1),
                            perf_mode=DR)
                    return ps, tts

                def proj_kv(tt):
                    ps, tts = proj(2, tt, f"projkv_{tt}")
                    kbf = pbfp.tile([128, TT], BF, tag="pbf", name=f"kbf{tt}")
                    nc.vector.tensor_copy(kbf[0:64, :], ps[0:64, :])
                    # v = ps/16
                    nc.vector.tensor_scalar_mul(vT[:, tts], ps[64:128, :],
                                                1.0 / 16.0)
                    rope(kbf, k2[0:64, tts], 64, tts, cosk_sb, sink_sb, "k")
                    nc.gpsimd.tensor_copy(k2[64:128, tts], k2[0:64, tts])
                    for c in range(4 * tt, 4 * tt + 4):
                        pt = psP.tile([128, 64], BF, tag="proj", name=f"vt{c}")
                        nc.tensor.transpose(pt[:], vT[:, bass.ts(c, 128)],
                                            ident[:])
                        nc.vector.tensor_copy(vaug[:, c, 0:64], pt[:])

                def proj_q(rc, tt):
                    ps, tts = proj(rc, tt, f"projq{rc}_{tt}")
                    pbf = pbfp.tile([128, TT], BF, tag="pbf",
                                    name=f"qbf{rc}_{tt}")
                    nc.vector.tensor_copy(pbf[:], ps[:])
                    rope(pbf, qrot[rc][:, tts], 128, tts, cos_sb, sin_sb, "q")

                def outproj_tile(tch, ht, last=False):
                    tcs = bass.ts(tch, 128)
                    hts = bass.ts(ht, TT)
                    po = psP.tile([128, TT], F32, tag="proj",
                                  name=f"po{tch}_{ht}")
                    terms = [(ahi[:, :, tcs], wohi_sb[:, :, hts]),
                             (ahi[:, :, tcs], wolo_sb[:, :, hts]),
                             (alo[:, :, tcs], wohi_sb[:, :, hts])]
                    for si, (a, w) in enumerate(terms):
                        nc.tensor.matmul(
                            po[:], a, w,
                            start=(si == 0), stop=(si == len(terms) - 1),
                            perf_mode=DR)
                    ob = op_.tile([128, TT], BF, tag="ob")
                    if last:
                        nc.scalar.copy(ob[:], po[:])
                    else:
                        nc.vector.tensor_copy(ob[:], po[:])
                    nc.sync.dma_start(out[tcs, hts], ob[:])

                # ---- filler queue
                filler = collections.deque()

                def pump(n=1):
                    for _ in range(n):
                        if not filler:
                            return
                        filler.popleft()()

                def attention_head(pair, h2, qt):
                    qts = bass.ts(qt, TT)
                    qrows = slice(64 * h2, 64 * h2 + 64)
                    pacc = psA.tile([65, TT], F32, tag="att",
                                    name=f"att{pair}_{h2}_{qt}")
                    pending = None
                    for cp in range(NCP):
                        sc = psS.tile([128, 2 * TT], F32, tag="sc",
                                      name=f"sc{pair}_{h2}_{qt}_{cp}")
                        for j in range(2):
                            c = 2 * cp + j
                            nc.tensor.matmul(
                                sc[:, bass.ts(j, TT)],
                                k2[qrows, bass.ts(c, 128)],
                                qrot[pair][qrows, qts],
                                start=True, stop=True,
                                tile_position=(64 * h2, 0))
                        ex = ep.tile([128, 2 * TT], BF, tag="exp")
                        nc.scalar.activation(ex[:], sc[:], AF.Exp)
                        pump(1)
                        if pending is not None:
                            pex, pcp = pending
                            for j in range(2):
                                c = 2 * pcp + j
                                nc.tensor.matmul(
                                    pacc[:], vaug[:, c, :],
                                    pex[:, bass.ts(j, TT)],
                                    start=(c == 0), stop=False)
                        pending = (ex, cp)
                        yield
                    pex, pcp = pending
                    for j in range(2):
                        c = 2 * pcp + j
                        nc.tensor.matmul(
                            pacc[:], vaug[:, c, :], pex[:, bass.ts(j, TT)],
                            start=False, stop=(c == KC - 1))
                    # normalize (den staged via SBUF: PSUM-recip broken on HW)
                    den = np_.tile([1, TT], F32, tag="den")
                    nc.vector.tensor_copy(den[:], pacc[64:65, :])
                    rec = np_.tile([1, TT], F32, tag="rec")
                    nc.vector.reciprocal_approx_fast(rec[:], den[:])
                    bc = np_.tile([64, TT], F32, tag="bc")
                    nc.gpsimd.partition_broadcast(bc[:], rec[:])
                    nc.vector.tensor_tensor(
                        anorm[qrows, pair, qts], pacc[0:64, :], bc[:], MULT)
                    # fp8 hi/lo for the DR outproj (Pool, SBUF-only)
                    nc.gpsimd.tensor_copy(ahi[qrows, pair, qts],
                                          anorm[qrows, pair, qts])
                    nc.gpsimd.tensor_tensor(
                        alo[qrows, pair, qts],
                        anorm[qrows, pair, qts],
                        ahi[qrows, pair, qts],
                        SUB)

                # ---- master schedule
                proj_kv(0)
                proj_q(0, 0)
                filler.append(lambda: proj_kv(1))
                filler.append(lambda: proj_q(1, 0))
                filler.append(lambda: proj_kv(2))
                filler.append(lambda: proj_kv(3))
                for tt in range(1, NTT):
                    for rc in range(2):
                        filler.append(
                            lambda rc=rc, tt=tt: proj_q(rc, tt))

                heads = [(pair, h2, qt)
                         for qt in range(NTT)
                         for pair in range(2)
                         for h2 in range(2)]

                def head_done(i):
                    if i % 4 == 3:
                        qt = heads[i][2]
                        last = i == len(heads) - 1
                        for tch in range(4 * qt, 4 * qt + 4):
                            for ht in range(2):
                                filler.append(
                                    lambda tch=tch, ht=ht, last=last:
                                    outproj_tile(tch, ht, last=last))

                nxt = 0

                def start_next():
                    nonlocal nxt
                    if nxt >= len(heads):
                        return None
                    g = attention_head(*heads[nxt])
                    nxt += 1
                    return (nxt - 1, g)

                slots = [start_next(), start_next()]
                while any(slots):
                    for si in range(2):
                        if slots[si] is None:
                            continue
                        i, g = slots[si]
                        try:
                            next(g)
                        except StopIteration:
                            head_done(i)
                            slots[si] = start_next()
                while filler:
                    pump(1)
    nc.finalize()
    return nc


def _get_nc():
    global _nc_cache
    if _nc_cache is None:
        _nc_cache = _build_bass()
    return _nc_cache


def _hilo(x):
    hi = x.astype(_E4)
    lo = (x - hi.astype(np.float32)).astype(_E5)
    return hi, lo


def _shard_inputs(hidden_states, cos, sin, w_qkv, w_o):
    """Build per-core input maps. Core c = (b = c // 4, g = c % 4)."""
    cosT = cos.T.astype(np.float32)                                # [64, S]
    sinT = sin.T.astype(np.float32)
    sinmod = np.concatenate([-sinT[0:32], sinT[32:64]], axis=0)
    qs = SCALE / 16.0
    cos2 = np.ascontiguousarray(
        np.concatenate([cosT, cosT], axis=0) * qs).astype(_BF16)
    sin2 = np.ascontiguousarray(
        np.concatenate([sinmod, sinmod], axis=0) * qs).astype(_BF16)
    cosk = np.ascontiguousarray(cosT / 16.0).astype(_BF16)
    sink = np.ascontiguousarray(sinmod / 16.0).astype(_BF16)

    # h packed [p, j, i, t]: feature 256j + 128i + p
    hsplit = []
    for b in range(B):
        ht = hidden_states[b].T.astype(np.float32)                 # [1024, S]
        hp = np.ascontiguousarray(
            ht.reshape(4, 2, 128, S).transpose(2, 0, 1, 3))        # [128,4,2,S]
        hsplit.append(_hilo(hp))
    in_maps = []
    for c in range(NCORES):
        b, g = divmod(c, 4)
        q_rows = w_qkv[256 * g: 256 * g + 256]
        k_rows = w_qkv[1024 + 64 * g: 1024 + 64 * g + 64]
        v_rows = w_qkv[1280 + 64 * g: 1280 + 64 * g + 64]
        wqk = np.concatenate([q_rows, k_rows, v_rows], axis=0)     # [384, 1024]
        # x16 into fp8 range; [p, rc, j, i, m] with h = 256j+128i+p
        wqkT = (wqk.T * 16.0).astype(np.float32)                   # [1024, 384]
        wpk = np.ascontiguousarray(
            wqkT.reshape(4, 2, 128, 3, 128).transpose(2, 3, 0, 1, 4))
        whi_a, wlo_a = _hilo(wpk)
        woTf = (w_o[:, 256 * g: 256 * g + 256].T * 16.0).astype(np.float32)
        wo_pk = np.ascontiguousarray(
            woTf.reshape(2, 128, HID).transpose(1, 0, 2))          # [128,2,HID]
        wohi_a, wolo_a = _hilo(wo_pk)
        in_maps.append(
            {
                "hhi": hsplit[b][0],
                "hlo": hsplit[b][1],
                "whi": whi_a,
                "wlo": wlo_a,
                "wohi": wohi_a,
                "wolo": wolo_a,
                "cosd": cos2,
                "sind": sin2,
                "coskd": cosk,
                "sinkd": sink,
            }
        )
    return in_maps


def _run(inputs, **spmd_kwargs):
    from concourse.bass_utils import run_bass_kernel_spmd

    nc = _get_nc()
    in_maps = _shard_inputs(**inputs)
    res = run_bass_kernel_spmd(
        nc, in_maps, core_ids=list(range(NCORES)), **spmd_kwargs
    )
    outs = []
    for b in range(B):
        acc = res.results[4 * b]["out"].astype(np.float32)
        for g in range(1, 4):
            acc = acc + res.results[4 * b + g]["out"].astype(np.float32)
        outs.append(acc * OUT_SCALE)
    return np.stack(outs, axis=0), res


def kernel(**inputs):
    out, _ = _run(inputs)
    return out


# revision 9
# speedup vs baseline: 1.3414x; 1.0142x over previous
"""GQA attention layer on 8 Trainium2 NeuronCores — v6.

v5 + fp8 hi/lo DoubleRow for both projections:
- QKV proj: host decomposes h and w_qkv into e4m3 hi + e5m2 lo; each K=256
  block is 3 DoubleRow matmuls (hi*hi + hi*lo + lo*hi) at half the PE cost
  of bf16, with ~bf16 accuracy (hi+lo carries ~14 mantissa bits).
- out proj: Pool computes anorm hi/lo from the bf16 normalize result;
  w_o decomposed on host. 3 DR matmuls replace 2 bf16 ones per tile.
Scaling: weights x16 into fp8 range, q rope tables fold SCALE/16, k tables
fold 1/16, V folds 1/16 at the vT copy, vaug ones-column = 1/32 so
anorm = 32*attn (fp8-friendly), w_o x16 -> output is 512x; host divides.
"""

import collections

import numpy as np
import ml_dtypes

B, S, HID = 2, 2048, 1024
NH, NKV, D = 16, 4, 64
SCALE = float(D ** -0.5)
NCORES = 8
TT = 512          # token tile
NTT = S // TT     # 4
KC = S // 128     # 16 key chunks
NCP = KC // 2     # 8 chunk pairs
OUT_SCALE = 1.0 / 512.0

_BF16 = ml_dtypes.bfloat16
_E4 = ml_dtypes.float8_e4m3
_E5 = ml_dtypes.float8_e5m2

_nc_cache = None


def _build_bass():
    import concourse.bass as bass
    import concourse.mybir as mybir
    import concourse.tile as tile
    from concourse import bacc
    from concourse.masks import make_identity

    BF = mybir.dt.bfloat16
    F32 = mybir.dt.float32
    I16 = mybir.dt.int16
    E4 = mybir.dt.float8e4
    E5 = mybir.dt.float8e5
    AF = mybir.ActivationFunctionType
    MULT = mybir.AluOpType.mult
    ADD = mybir.AluOpType.add
    SUB = mybir.AluOpType.subtract
    DR = mybir.MatmulPerfMode.DoubleRow
    # Schraudolph exp for offloaded tiles: bitcast(int16(s*A + B)) ~ exp(s)
    SCH_A = 184.66496280558537 * SCALE   # 128/ln2, scores carry 1/SCALE
    SCH_B = 16256.0 - 5.75 + 0.5         # bias center + truncation fix

    nc = bacc.Bacc()
    # h split hi/lo, packed [p, j, i, t]: h feature 256j + 128i + p
    hhi = nc.dram_tensor("hhi", (128, 4, 2, S), E4, kind="ExternalInput")
    hlo = nc.dram_tensor("hlo", (128, 4, 2, S), E5, kind="ExternalInput")
    # wqk split hi/lo, packed [p, rc, j, i, m]
    whi = nc.dram_tensor("whi", (128, 3, 4, 2, 128), E4, kind="ExternalInput")
    wlo = nc.dram_tensor("wlo", (128, 3, 4, 2, 128), E5, kind="ExternalInput")
    wohi = nc.dram_tensor("wohi", (128, 2, HID), E4, kind="ExternalInput")
    wolo = nc.dram_tensor("wolo", (128, 2, HID), E5, kind="ExternalInput")
    # shared q/k rope tables (/16); SCALE is applied by the exp activation
    cosd = nc.dram_tensor("cosd", (64, S), BF, kind="ExternalInput")
    sind = nc.dram_tensor("sind", (64, S), BF, kind="ExternalInput")
    out = nc.dram_tensor("out", (S, HID), BF, kind="ExternalOutput")

    with tile.TileContext(nc) as tc:
        with (
            tc.tile_pool(name="persist", bufs=1) as pp,
            tc.tile_pool(name="pbfp", bufs=3) as pbfp,
            tc.tile_pool(name="rope", bufs=3) as rp,
            tc.tile_pool(name="exps", bufs=6) as ep,
            tc.tile_pool(name="norm", bufs=4) as np_,
            tc.tile_pool(name="outsb", bufs=4) as op_,
        ):
            # ---- persistent SBUF tiles + input loads, kv-first order
            hhi_sb = pp.tile([128, 4, 2, S], E4, tag="hhi_sb")
            hlo_sb = pp.tile([128, 4, 2, S], E5, tag="hlo_sb")
            whi_sb = pp.tile([128, 3, 4, 2, 128], E4, tag="whi_sb")
            wlo_sb = pp.tile([128, 3, 4, 2, 128], E5, tag="wlo_sb")
            wohi_sb = pp.tile([128, 2, HID], E4, tag="wohi_sb")
            wolo_sb = pp.tile([128, 2, HID], E5, tag="wolo_sb")
            # [128, S]: rows 0:64 DMA'd, rows 64:128 duplicated on-device
            cos_sb = pp.tile([128, S], BF, tag="cos_sb")
            sin_sb = pp.tile([128, S], BF, tag="sin_sb")

            def h_slice(t0, t1):
                for hd, hs in ((hhi, hhi_sb), (hlo, hlo_sb)):
                    nc.sync.dma_start(hs[:, :, :, t0:t1], hd[:, :, :, t0:t1])

            nc.sync.dma_start(whi_sb[:, 2], whi[:, 2])
            nc.sync.dma_start(hhi_sb[:, :, :, 0:TT], hhi[:, :, :, 0:TT])
            nc.sync.dma_start(whi_sb[:, 0:2], whi[:, 0:2])
            nc.sync.dma_start(wlo_sb[:, 2], wlo[:, 2])
            nc.sync.dma_start(hlo_sb[:, :, :, 0:TT], hlo[:, :, :, 0:TT])
            nc.sync.dma_start(wlo_sb[:, 0:2], wlo[:, 0:2])
            for tt in range(NTT):
                tts_ = bass.ts(tt, TT)
                if tt > 0:
                    h_slice(tt * TT, (tt + 1) * TT)
                nc.sync.dma_start(cos_sb[0:64, tts_], cosd[:, tts_])
                nc.sync.dma_start(sin_sb[0:64, tts_], sind[:, tts_])
                # Pool duplicates the tables onto partitions 64:128
                nc.gpsimd.tensor_copy(cos_sb[64:128, tts_],
                                      cos_sb[0:64, tts_])
                nc.gpsimd.tensor_copy(sin_sb[64:128, tts_],
                                      sin_sb[0:64, tts_])
            nc.sync.dma_start(wohi_sb[:], wohi[:])
            nc.sync.dma_start(wolo_sb[:], wolo[:])

            ident = pp.tile([64, 64], BF, tag="ident")
            make_identity(nc, ident[:])
            warm = pp.tile([1, 8], F32, tag="warm")
            nc.any.memset(warm[:], 0.0)
            nc.scalar.activation(warm[:], warm[:], AF.Exp)
            # keep the tensor engine busy on junk matmuls while the first
            # h/w DMAs land, so the p-state ramp completes before real work
            wa = pp.tile([128, TT], BF, tag="wa")
            nc.gpsimd.memset(wa[:], 0.5)

            qrot = [pp.tile([128, S], BF, tag=f"qrot{p}", name=f"qrot{p}")
                    for p in range(2)]
            k2 = pp.tile([128, S], BF, tag="k2")
            vT = pp.tile([64, S], BF, tag="vT")
            vaug = pp.tile([128, KC, 65], BF, tag="vaug")
            nc.any.memset(vaug[:], 1.0 / 32.0)
            # anorm = 32*attn: bf16 full + fp8 hi/lo for the DR outproj,
            # packed [p, oc(=pair), t]
            anorm = pp.tile([128, 2, S], BF, tag="anorm")
            ahi = pp.tile([128, 2, S], E4, tag="ahi")
            alo = pp.tile([128, 2, S], E5, tag="alo")

            with (
                tc.tile_pool(name="psP", bufs=2, space="PSUM") as psP,
                tc.tile_pool(name="psS", bufs=2, space="PSUM") as psS,
                tc.tile_pool(name="psA", bufs=2, space="PSUM") as psA,
            ):

                def rope(pbf, dest, rows, tts, tag):
                    """Pool builds the 32-block-swapped copy; DVE runs
                    same-partition bf16 2x-mode multiply/add ops against the
                    compact shared [64, S] tables."""
                    sg = rp.tile([128, TT], BF, tag=f"sg{tag}")
                    for blk in range(rows // 32):
                        src = blk ^ 1
                        nc.gpsimd.tensor_copy(
                            sg[32 * blk: 32 * blk + 32, :],
                            pbf[32 * src: 32 * src + 32, :])
                    t1 = rp.tile([128, TT], BF, tag=f"t1{tag}")
                    rt = rp.tile([128, TT], BF, tag=f"rt{tag}")
                    nc.vector.tensor_tensor(
                        t1[0:rows, :], pbf[0:rows, :], cos_sb[0:rows, tts],
                        MULT)
                    nc.vector.tensor_tensor(
                        rt[0:rows, :], sg[0:rows, :], sin_sb[0:rows, tts],
                        MULT)
                    nc.vector.tensor_tensor(
                        dest, t1[0:rows, :], rt[0:rows, :], ADD)

                def proj(rc, tt, name, slot):
                    """hi/lo DoubleRow projection: 12 accumulating DR matmuls
                    (4 K=256 blocks x {hi*hi, hi*lo, lo*hi}). slot borrows an
                    idle PSUM ring early in the prologue."""
                    tts = bass.ts(tt, TT)
                    pool, tag = slot
                    if tag == "sc":
                        ps = pool.tile([128, 2 * TT], F32, tag="sc",
                                       name=name)[:, 0:TT]
                    else:
                        ps = pool.tile([128, TT], F32, tag=tag, name=name)
                    steps = []
                    for j in range(4):
                        steps.append((whi_sb[:, rc, j], hhi_sb[:, j, :, tts]))
                    for j in range(4):
                        steps.append((whi_sb[:, rc, j], hlo_sb[:, j, :, tts]))
                        steps.append((wlo_sb[:, rc, j], hhi_sb[:, j, :, tts]))
                    for si, (w, x) in enumerate(steps):
                        nc.tensor.matmul(
                            ps[:], w, x,
                            start=(si == 0), stop=(si == len(steps) - 1),
                            perf_mode=DR)
                    return ps, tts

                def proj_kv(tt, slot=(psP, "proj"), act_copy=False):
                    ps, tts = proj(2, tt, f"projkv_{tt}", slot)
                    kbf = pbfp.tile([128, TT], BF, tag="pbf", name=f"kbf{tt}")
                    if act_copy:
                        nc.scalar.copy(kbf[0:64, :], ps[0:64, :])
                        nc.scalar.copy(vT[:, tts], ps[64:128, :])
                        nc.vector.tensor_scalar_mul(vT[:, tts], vT[:, tts],
                                                    1.0 / 16.0)
                    else:
                        nc.vector.tensor_copy(kbf[0:64, :], ps[0:64, :])
                        # v = ps/16
                        nc.vector.tensor_scalar_mul(vT[:, tts], ps[64:128, :],
                                                    1.0 / 16.0)
                    rope(kbf, k2[0:64, tts], 64, tts, "k")
                    nc.gpsimd.tensor_copy(k2[64:128, tts], k2[0:64, tts])

                def transp(tt, slot=(psP, "proj")):
                    pool, tag = slot
                    pt = pool.tile([128, 4, 64], BF, tag=tag,
                                   name=f"vt{tt}")
                    for ci in range(4):
                        c = 4 * tt + ci
                        nc.tensor.transpose(pt[:, ci, :],
                                            vT[:, bass.ts(c, 128)], ident[:])
                    nc.vector.tensor_copy(vaug[:, 4 * tt: 4 * tt + 4, 0:64],
                                          pt[:])

                def proj_q(rc, tt, slot=(psP, "proj"), act_copy=False):
                    ps, tts = proj(rc, tt, f"projq{rc}_{tt}", slot)
                    pbf = pbfp.tile([128, TT], BF, tag="pbf",
                                    name=f"qbf{rc}_{tt}")
                    nc.vector.tensor_copy(pbf[:], ps[:])
                    rope(pbf, qrot[rc][:, tts], 128, tts, "q")

                def outproj_tile(tch, ht, last=False):
                    tcs = bass.ts(tch, 128)
                    hts = bass.ts(ht, TT)
                    po = psP.tile([128, TT], F32, tag="proj",
                                  name=f"po{tch}_{ht}")
                    terms = [(ahi[:, :, tcs], wohi_sb[:, :, hts]),
                             (ahi[:, :, tcs], wolo_sb[:, :, hts]),
                             (alo[:, :, tcs], wohi_sb[:, :, hts])]
                    for si, (a, w) in enumerate(terms):
                        nc.tensor.matmul(
                            po[:], a, w,
                            start=(si == 0), stop=(si == len(terms) - 1),
                            perf_mode=DR)
                    ob = op_.tile([128, TT], BF, tag="ob")
                    if last and (tch + ht) % 2 == 0:
                        nc.scalar.copy(ob[:], po[:])
                    else:
                        nc.vector.tensor_copy(ob[:], po[:])
                    nc.sync.dma_start(out[tcs, hts], ob[:])

                # ---- filler queue
                filler = collections.deque()

                def pump(n=1):
                    for _ in range(n):
                        if not filler:
                            return
                        filler.popleft()()

                def attention_head(pair, h2, qt, offload=True,
                                   last_head=False):
                    qts = bass.ts(qt, TT)
                    qrows = slice(64 * h2, 64 * h2 + 64)
                    pacc = psA.tile([65, TT], F32, tag="att",
                                    name=f"att{pair}_{h2}_{qt}")
                    pending = collections.deque()

                    def drain_pending(keep):
                        while len(pending) > keep:
                            pex, pcp = pending.popleft()
                            for j in range(2):
                                c = 2 * pcp + j
                                nc.tensor.matmul(
                                    pacc[:], vaug[:, c, :],
                                    pex[:, 512 * j: 512 * j + 512],
                                    start=(c == 0), stop=(c == KC - 1))

                    for cp in range(NCP):
                        sc = psS.tile([128, 2 * TT], F32, tag="sc",
                                      name=f"sc{pair}_{h2}_{qt}_{cp}")
                        for j in range(2):
                            c = 2 * cp + j
                            nc.tensor.matmul(
                                sc[:, bass.ts(j, TT)],
                                k2[qrows, bass.ts(c, 128)],
                                qrot[pair][qrows, qts],
                                start=True, stop=True,
                                tile_position=(64 * h2, 0))
                        if cp == 3 and offload:
                            # offload this tile's exp to DVE (Schraudolph);
                            # the ~2% approx error on 1/8 of the keys is
                            # within budget and relieves the pacing engine
                            exi = ep.tile([128, 2 * TT], I16, tag="exps")
                            nc.vector.tensor_scalar(exi[:], sc[:],
                                                    SCH_A, SCH_B, MULT, ADD)
                            ex = exi[:].bitcast(BF)
                        else:
                            ext = ep.tile([128, 2 * TT], BF, tag="exp")
                            nc.scalar.activation(ext[:], sc[:], AF.Exp,
                                                 scale=SCALE)
                            ex = ext[:]
                        pending.append((ex, cp))
                        # the DVE-produced (offloaded) tile gets an extra
                        # cpair of slack before its attnV is issued
                        drain_pending(2 if cp in (3, 4) else 1)
                        yield
                    drain_pending(0)
                    # copy pacc to SBUF bf16 immediately -> psA bank freed in
                    # one op; normalize runs SBUF-side in bf16 2x-mode ops
                    att = np_.tile([64, TT], F32, tag="att_sb")
                    nc.vector.tensor_copy(att[:], pacc[0:64, :])
                    # den staged to a partition-0 tile: reciprocal reading an
                    # SBUF slice at base partition 64 returns garbage on HW
                    den = np_.tile([1, TT], F32, tag="den")
                    nc.vector.tensor_copy(den[:], pacc[64:65, :])
                    rec = np_.tile([1, TT], F32, tag="rec")
                    nc.vector.reciprocal_approx_fast(rec[:], den[:])
                    bc = np_.tile([64, TT], F32, tag="bc")
                    nc.gpsimd.partition_broadcast(bc[:], rec[:])
                    nc.vector.tensor_tensor(
                        anorm[qrows, pair, qts], att[0:64, :], bc[:], MULT)
                    # fp8 hi/lo for the DR outproj (Pool; DVE for the last
                    # head to shorten the tail chain)
                    eng = nc.vector if last_head else nc.gpsimd
                    eng.tensor_copy(ahi[qrows, pair, qts],
                                    anorm[qrows, pair, qts])
                    eng.tensor_tensor(
                        alo[qrows, pair, qts],
                        anorm[qrows, pair, qts],
                        ahi[qrows, pair, qts],
                        SUB)

                # ---- master schedule: kv0/q00 up front with Activation-
                # assisted copies (exp idle), V-transpose 0 borrows the psA
                # ring; later h tiles are DMA-gated so they pump as filler.
                for wi in range(9):
                    wps = psA.tile([128, TT], F32, tag="att",
                                   name=f"warmmm{wi}")
                    nc.tensor.matmul(wps[:, 0:256], wa[:, 0:128],
                                     wa[:, 0:256], start=True, stop=True)
                proj_kv(0, slot=(psP, "proj"), act_copy=True)
                proj_q(0, 0, slot=(psP, "proj"), act_copy=True)
                transp(0, slot=(psA, "att"))
                proj_kv(1)
                filler.append(lambda: proj_kv(2))
                filler.append(lambda: transp(1))
                filler.append(lambda: proj_q(1, 0, slot=(psA, "att")))
                filler.append(lambda: proj_kv(3))
                filler.append(lambda: transp(2))
                filler.append(lambda: transp(3))
                for tt in range(1, NTT):
                    for rc in range(2):
                        filler.append(
                            lambda rc=rc, tt=tt: proj_q(rc, tt))

                heads = [(pair, h2, qt)
                         for qt in range(NTT)
                         for pair in range(2)
                         for h2 in range(2)]

                def head_done(i):
                    if i % 4 == 3:
                        qt = heads[i][2]
                        last = i == len(heads) - 1
                        for tch in range(4 * qt, 4 * qt + 4):
                            for ht in range(2):
                                filler.append(
                                    lambda tch=tch, ht=ht, last=last:
                                    outproj_tile(tch, ht, last=last))

                nxt = 0

                def start_next():
                    nonlocal nxt
                    if nxt >= len(heads):
                        return None
                    g = attention_head(*heads[nxt], offload=(nxt >= 4),
                                       last_head=(nxt == len(heads) - 1))
                    nxt += 1
                    return (nxt - 1, g)

                nproj_fill = len(filler)
                slots = [start_next(), None]
                stagger = 6
                step = 0
                while any(slots):
                    for si in range(2):
                        if slots[si] is None:
                            continue
                        i, g = slots[si]
                        try:
                            next(g)
                            # drain the projection fillers at double rate so
                            # their PSUM->rope chains stay ahead of attention
                            pump(2 if step < nproj_fill else 1)
                            step += 1
                            if stagger is not None:
                                stagger -= 1
                                if stagger == 0:
                                    slots[1] = start_next()
                                    stagger = None
                        except StopIteration:
                            head_done(i)
                            slots[si] = start_next()
                while filler:
                    pump(1)
    nc.finalize()
    return nc


def _get_nc():
    global _nc_cache
    if _nc_cache is None:
        _nc_cache = _build_bass()
    return _nc_cache


def _hilo(x):
    hi = x.astype(_E4)
    lo = (x - hi.astype(np.float32)).astype(_E5)
    return hi, lo


def _shard_inputs(hidden_states, cos, sin, w_qkv, w_o):
    """Build per-core input maps. Core c = (b = c // 4, g = c % 4)."""
    cosT = cos.T.astype(np.float32)                                # [64, S]
    sinT = sin.T.astype(np.float32)
    sinmod = np.concatenate([-sinT[0:32], sinT[32:64]], axis=0)
    cosc = np.ascontiguousarray(cosT / 16.0).astype(_BF16)
    sinc = np.ascontiguousarray(sinmod / 16.0).astype(_BF16)

    # h packed [p, j, i, t]: feature 256j + 128i + p
    hsplit = []
    for b in range(B):
        ht = hidden_states[b].T.astype(np.float32)                 # [1024, S]
        hp = np.ascontiguousarray(
            ht.reshape(4, 2, 128, S).transpose(2, 0, 1, 3))        # [128,4,2,S]
        hsplit.append(_hilo(hp))
    in_maps = []
    for c in range(NCORES):
        b, g = divmod(c, 4)
        q_rows = w_qkv[256 * g: 256 * g + 256]
        k_rows = w_qkv[1024 + 64 * g: 1024 + 64 * g + 64]
        v_rows = w_qkv[1280 + 64 * g: 1280 + 64 * g + 64]
        wqk = np.concatenate([q_rows, k_rows, v_rows], axis=0)     # [384, 1024]
        # x16 into fp8 range; [p, rc, j, i, m] with h = 256j+128i+p
        wqkT = (wqk.T * 16.0).astype(np.float32)                   # [1024, 384]
        wpk = np.ascontiguousarray(
            wqkT.reshape(4, 2, 128, 3, 128).transpose(2, 3, 0, 1, 4))
        whi_a, wlo_a = _hilo(wpk)
        woTf = (w_o[:, 256 * g: 256 * g + 256].T * 16.0).astype(np.float32)
        wo_pk = np.ascontiguousarray(
            woTf.reshape(2, 128, HID).transpose(1, 0, 2))          # [128,2,HID]
        wohi_a, wolo_a = _hilo(wo_pk)
        in_maps.append(
            {
                "hhi": hsplit[b][0],
                "hlo": hsplit[b][1],
                "whi": whi_a,
                "wlo": wlo_a,
                "wohi": wohi_a,
                "wolo": wolo_a,
                "cosd": cosc,
                "sind": sinc,
            }
        )
    return in_maps


def _run(inputs, **spmd_kwargs):
    from concourse.bass_utils import run_bass_kernel_spmd

    nc = _get_nc()
    in_maps = _shard_inputs(**inputs)
    res = run_bass_kernel_spmd(
        nc, in_maps, core_ids=list(range(NCORES)), **spmd_kwargs
    )
    outs = []
    for b in range(B):
        acc = res.results[4 * b]["out"].astype(np.float32)
        for g in range(1, 4):
            acc = acc + res.results[4 * b + g]["out"].astype(np.float32)
        outs.append(acc * OUT_SCALE)
    return np.stack(outs, axis=0), res


def kernel(**inputs):
    out, _ = _run(inputs)
    return out


# revision 10
# speedup vs baseline: 1.3759x; 1.0257x over previous
"""GQA attention layer on 8 Trainium2 NeuronCores — v6.

v5 + fp8 hi/lo DoubleRow for both projections:
- QKV proj: host decomposes h and w_qkv into e4m3 hi + e5m2 lo; each K=256
  block is 3 DoubleRow matmuls (hi*hi + hi*lo + lo*hi) at half the PE cost
  of bf16, with ~bf16 accuracy (hi+lo carries ~14 mantissa bits).
- out proj: Pool computes anorm hi/lo from the bf16 normalize result;
  w_o decomposed on host. 3 DR matmuls replace 2 bf16 ones per tile.
Scaling: weights x16 into fp8 range, q rope tables fold SCALE/16, k tables
fold 1/16, V folds 1/16 at the vT copy, vaug ones-column = 1/32 so
anorm = 32*attn (fp8-friendly), w_o x16 -> output is 512x; host divides.
"""

import collections

import numpy as np
import ml_dtypes

B, S, HID = 2, 2048, 1024
NH, NKV, D = 16, 4, 64
SCALE = float(D ** -0.5)
NCORES = 8
TT = 512          # token tile
NTT = S // TT     # 4
KC = S // 128     # 16 key chunks
NCP = KC // 2     # 8 chunk pairs
OUT_SCALE = 1.0 / 512.0

_BF16 = ml_dtypes.bfloat16
_E4 = ml_dtypes.float8_e4m3
_E5 = ml_dtypes.float8_e5m2

_nc_cache = None


def _build_bass():
    import concourse.bass as bass
    import concourse.mybir as mybir
    import concourse.tile as tile
    from concourse import bacc
    from concourse.masks import make_identity

    BF = mybir.dt.bfloat16
    F32 = mybir.dt.float32
    I16 = mybir.dt.int16
    E4 = mybir.dt.float8e4
    E5 = mybir.dt.float8e5
    AF = mybir.ActivationFunctionType
    MULT = mybir.AluOpType.mult
    ADD = mybir.AluOpType.add
    SUB = mybir.AluOpType.subtract
    DR = mybir.MatmulPerfMode.DoubleRow
    # Schraudolph exp for offloaded tiles: bitcast(int16(s*A + B)) ~ exp(s)
    SCH_A = 184.66496280558537 * SCALE   # 128/ln2, scores carry 1/SCALE
    SCH_B = 16256.0 - 5.75 + 0.5         # bias center + truncation fix

    nc = bacc.Bacc()
    # h split hi/lo, packed [p, j, i, t]: h feature 256j + 128i + p
    hhi = nc.dram_tensor("hhi", (128, 4, 2, S), E4, kind="ExternalInput")
    hlo = nc.dram_tensor("hlo", (128, 4, 2, S), E5, kind="ExternalInput")
    # wqk split hi/lo, packed [p, rc, j, i, m]
    whi = nc.dram_tensor("whi", (128, 3, 4, 2, 128), E4, kind="ExternalInput")
    wlo = nc.dram_tensor("wlo", (128, 3, 4, 2, 128), E5, kind="ExternalInput")
    wohi = nc.dram_tensor("wohi", (128, 2, HID), E4, kind="ExternalInput")
    wolo = nc.dram_tensor("wolo", (128, 2, HID), E5, kind="ExternalInput")
    # shared q/k rope tables (/16); SCALE is applied by the exp activation
    cosd = nc.dram_tensor("cosd", (64, S), BF, kind="ExternalInput")
    sind = nc.dram_tensor("sind", (64, S), BF, kind="ExternalInput")
    out = nc.dram_tensor("out", (S, HID), BF, kind="ExternalOutput")

    with tile.TileContext(nc) as tc:
        with (
            tc.tile_pool(name="persist", bufs=1) as pp,
            tc.tile_pool(name="pbfp", bufs=3) as pbfp,
            tc.tile_pool(name="rope", bufs=3) as rp,
            tc.tile_pool(name="exps", bufs=6) as ep,
            tc.tile_pool(name="norm", bufs=4) as np_,
            tc.tile_pool(name="outsb", bufs=4) as op_,
        ):
            # ---- persistent SBUF tiles + input loads, kv-first order
            hhi_sb = pp.tile([128, 4, 2, S], E4, tag="hhi_sb")
            hlo_sb = pp.tile([128, 4, 2, S], E5, tag="hlo_sb")
            whi_sb = pp.tile([128, 3, 4, 2, 128], E4, tag="whi_sb")
            wlo_sb = pp.tile([128, 3, 4, 2, 128], E5, tag="wlo_sb")
            wohi_sb = pp.tile([128, 2, HID], E4, tag="wohi_sb")
            wolo_sb = pp.tile([128, 2, HID], E5, tag="wolo_sb")
            # [128, S]: rows 0:64 DMA'd, rows 64:128 duplicated on-device
            cos_sb = pp.tile([128, S], BF, tag="cos_sb")
            sin_sb = pp.tile([128, S], BF, tag="sin_sb")

            def h_slice(t0, t1):
                for hd, hs in ((hhi, hhi_sb), (hlo, hlo_sb)):
                    nc.sync.dma_start(hs[:, :, :, t0:t1], hd[:, :, :, t0:t1])

            nc.sync.dma_start(whi_sb[:, 2], whi[:, 2])
            nc.sync.dma_start(hhi_sb[:, :, :, 0:TT], hhi[:, :, :, 0:TT])
            nc.sync.dma_start(whi_sb[:, 0:2], whi[:, 0:2])
            nc.sync.dma_start(wlo_sb[:, 2], wlo[:, 2])
            nc.sync.dma_start(hlo_sb[:, :, :, 0:TT], hlo[:, :, :, 0:TT])
            nc.sync.dma_start(wlo_sb[:, 0:2], wlo[:, 0:2])
            for tt in range(NTT):
                tts_ = bass.ts(tt, TT)
                if tt > 0:
                    h_slice(tt * TT, (tt + 1) * TT)
                nc.sync.dma_start(cos_sb[0:64, tts_], cosd[:, tts_])
                nc.sync.dma_start(sin_sb[0:64, tts_], sind[:, tts_])
                # Pool duplicates the tables onto partitions 64:128
                nc.gpsimd.tensor_copy(cos_sb[64:128, tts_],
                                      cos_sb[0:64, tts_])
                nc.gpsimd.tensor_copy(sin_sb[64:128, tts_],
                                      sin_sb[0:64, tts_])
            nc.sync.dma_start(wohi_sb[:], wohi[:])
            nc.sync.dma_start(wolo_sb[:], wolo[:])

            ident = pp.tile([64, 64], BF, tag="ident")
            make_identity(nc, ident[:])
            warm = pp.tile([1, 8], F32, tag="warm")
            nc.any.memset(warm[:], 0.0)
            nc.scalar.activation(warm[:], warm[:], AF.Exp)
            # keep the tensor engine busy on junk matmuls while the first
            # h/w DMAs land, so the p-state ramp completes before real work
            wa = pp.tile([128, TT], BF, tag="wa")
            nc.gpsimd.memset(wa[:], 0.5)

            qrot = [pp.tile([128, S], BF, tag=f"qrot{p}", name=f"qrot{p}")
                    for p in range(2)]
            k2 = pp.tile([128, S], BF, tag="k2")
            vT = pp.tile([64, S], BF, tag="vT")
            vaug = pp.tile([128, KC, 65], BF, tag="vaug")
            nc.any.memset(vaug[:], 1.0 / 32.0)
            # anorm = 32*attn: bf16 full + fp8 hi/lo for the DR outproj,
            # packed [p, oc(=pair), t]
            anorm = pp.tile([128, 2, S], BF, tag="anorm")
            ahi = pp.tile([128, 2, S], E4, tag="ahi")
            alo = pp.tile([128, 2, S], E5, tag="alo")

            with (
                tc.tile_pool(name="psP", bufs=2, space="PSUM") as psP,
                tc.tile_pool(name="psS", bufs=2, space="PSUM") as psS,
                tc.tile_pool(name="psA", bufs=2, space="PSUM") as psA,
            ):

                def rope(pbf, dest, rows, tts, tag):
                    """Pool builds the 32-block-swapped copy; DVE runs
                    same-partition bf16 2x-mode multiply/add ops against the
                    compact shared [64, S] tables."""
                    sg = rp.tile([128, TT], BF, tag=f"sg{tag}")
                    for blk in range(rows // 32):
                        src = blk ^ 1
                        nc.gpsimd.tensor_copy(
                            sg[32 * blk: 32 * blk + 32, :],
                            pbf[32 * src: 32 * src + 32, :])
                    t1 = rp.tile([128, TT], BF, tag=f"t1{tag}")
                    rt = rp.tile([128, TT], BF, tag=f"rt{tag}")
                    nc.vector.tensor_tensor(
                        t1[0:rows, :], pbf[0:rows, :], cos_sb[0:rows, tts],
                        MULT)
                    nc.vector.tensor_tensor(
                        rt[0:rows, :], sg[0:rows, :], sin_sb[0:rows, tts],
                        MULT)
                    nc.vector.tensor_tensor(
                        dest, t1[0:rows, :], rt[0:rows, :], ADD)

                def proj(rc, tt, name, slot):
                    """hi/lo DoubleRow projection: 12 accumulating DR matmuls
                    (4 K=256 blocks x {hi*hi, hi*lo, lo*hi}). slot borrows an
                    idle PSUM ring early in the prologue."""
                    tts = bass.ts(tt, TT)
                    pool, tag = slot
                    if tag == "sc":
                        ps = pool.tile([128, 2 * TT], F32, tag="sc",
                                       name=name)[:, 0:TT]
                    else:
                        ps = pool.tile([128, TT], F32, tag=tag, name=name)
                    steps = []
                    for j in range(4):
                        steps.append((whi_sb[:, rc, j], hhi_sb[:, j, :, tts]))
                    for j in range(4):
                        steps.append((whi_sb[:, rc, j], hlo_sb[:, j, :, tts]))
                        steps.append((wlo_sb[:, rc, j], hhi_sb[:, j, :, tts]))
                    for si, (w, x) in enumerate(steps):
                        nc.tensor.matmul(
                            ps[:], w, x,
                            start=(si == 0), stop=(si == len(steps) - 1),
                            perf_mode=DR)
                    return ps, tts

                def proj_kv(tt, slot=(psP, "proj"), act_copy=False):
                    ps, tts = proj(2, tt, f"projkv_{tt}", slot)
                    kbf = pbfp.tile([128, TT], BF, tag="pbf", name=f"kbf{tt}")
                    if act_copy:
                        nc.scalar.copy(kbf[0:64, :], ps[0:64, :])
                        nc.scalar.copy(vT[:, tts], ps[64:128, :])
                        nc.vector.tensor_scalar_mul(vT[:, tts], vT[:, tts],
                                                    1.0 / 16.0)
                    else:
                        nc.vector.tensor_copy(kbf[0:64, :], ps[0:64, :])
                        # v = ps/16
                        nc.vector.tensor_scalar_mul(vT[:, tts], ps[64:128, :],
                                                    1.0 / 16.0)
                    rope(kbf, k2[0:64, tts], 64, tts, "k")
                    nc.gpsimd.tensor_copy(k2[64:128, tts], k2[0:64, tts])

                def transp(tt, slot=(psP, "proj")):
                    pool, tag = slot
                    pt = pool.tile([128, 4, 64], BF, tag=tag,
                                   name=f"vt{tt}")
                    for ci in range(4):
                        c = 4 * tt + ci
                        nc.tensor.transpose(pt[:, ci, :],
                                            vT[:, bass.ts(c, 128)], ident[:])
                    nc.vector.tensor_copy(vaug[:, 4 * tt: 4 * tt + 4, 0:64],
                                          pt[:])

                def proj_q(rc, tt, slot=(psP, "proj"), act_copy=False):
                    ps, tts = proj(rc, tt, f"projq{rc}_{tt}", slot)
                    pbf = pbfp.tile([128, TT], BF, tag="pbf",
                                    name=f"qbf{rc}_{tt}")
                    nc.vector.tensor_copy(pbf[:], ps[:])
                    rope(pbf, qrot[rc][:, tts], 128, tts, "q")

                def outproj_tile(tch, ht, last=False):
                    tcs = bass.ts(tch, 128)
                    hts = bass.ts(ht, TT)
                    po = psP.tile([128, TT], F32, tag="proj",
                                  name=f"po{tch}_{ht}")
                    terms = [(ahi[:, :, tcs], wohi_sb[:, :, hts]),
                             (ahi[:, :, tcs], wolo_sb[:, :, hts]),
                             (alo[:, :, tcs], wohi_sb[:, :, hts])]
                    for si, (a, w) in enumerate(terms):
                        nc.tensor.matmul(
                            po[:], a, w,
                            start=(si == 0), stop=(si == len(terms) - 1),
                            perf_mode=DR)
                    ob = op_.tile([128, TT], BF, tag="ob")
                    if (tch + ht) % 2 == 0:
                        nc.scalar.copy(ob[:], po[:])
                    else:
                        nc.vector.tensor_copy(ob[:], po[:])
                    nc.sync.dma_start(out[tcs, hts], ob[:])

                # ---- filler queue
                filler = collections.deque()

                def pump(n=1):
                    for _ in range(n):
                        if not filler:
                            return
                        filler.popleft()()

                def attention_head(pair, h2, qt, offload=True,
                                   last_head=False):
                    qts = bass.ts(qt, TT)
                    qrows = slice(64 * h2, 64 * h2 + 64)
                    pacc = psA.tile([65, TT], F32, tag="att",
                                    name=f"att{pair}_{h2}_{qt}")
                    pending = collections.deque()

                    def drain_pending(keep):
                        while len(pending) > keep:
                            pex, pcp = pending.popleft()
                            for j in range(2):
                                c = 2 * pcp + j
                                nc.tensor.matmul(
                                    pacc[:], vaug[:, c, :],
                                    pex[:, 512 * j: 512 * j + 512],
                                    start=(c == 0), stop=(c == KC - 1))

                    for cp in range(NCP):
                        sc = psS.tile([128, 2 * TT], F32, tag="sc",
                                      name=f"sc{pair}_{h2}_{qt}_{cp}")
                        for j in range(2):
                            c = 2 * cp + j
                            nc.tensor.matmul(
                                sc[:, bass.ts(j, TT)],
                                k2[qrows, bass.ts(c, 128)],
                                qrot[pair][qrows, qts],
                                start=True, stop=True,
                                tile_position=(64 * h2, 0))
                        if cp == 3 and offload:
                            # offload this tile's exp to DVE (Schraudolph);
                            # the ~2% approx error on 1/8 of the keys is
                            # within budget and relieves the pacing engine
                            exi = ep.tile([128, 2 * TT], I16, tag="exps")
                            nc.vector.tensor_scalar(exi[:], sc[:],
                                                    SCH_A, SCH_B, MULT, ADD)
                            ex = exi[:].bitcast(BF)
                        else:
                            ext = ep.tile([128, 2 * TT], BF, tag="exp")
                            nc.scalar.activation(ext[:], sc[:], AF.Exp,
                                                 scale=SCALE)
                            ex = ext[:]
                        pending.append((ex, cp))
                        # the DVE-produced (offloaded) tile gets an extra
                        # cpair of slack before its attnV is issued
                        drain_pending(2 if cp in (3, 4) else 1)
                        yield
                    drain_pending(0)
                    # copy pacc to SBUF bf16 immediately -> psA bank freed in
                    # one op; normalize runs SBUF-side in bf16 2x-mode ops
                    att = np_.tile([64, TT], F32, tag="att_sb")
                    nc.vector.tensor_copy(att[:], pacc[0:64, :])
                    # den staged to a partition-0 tile: reciprocal reading an
                    # SBUF slice at base partition 64 returns garbage on HW
                    den = np_.tile([1, TT], F32, tag="den")
                    nc.vector.tensor_copy(den[:], pacc[64:65, :])
                    rec = np_.tile([1, TT], F32, tag="rec")
                    nc.vector.reciprocal_approx_fast(rec[:], den[:])
                    bc = np_.tile([64, TT], F32, tag="bc")
                    nc.gpsimd.partition_broadcast(bc[:], rec[:])
                    nc.vector.tensor_tensor(
                        anorm[qrows, pair, qts], att[0:64, :], bc[:], MULT)
                    # fp8 hi/lo for the DR outproj (Pool; DVE for the last
                    # head to shorten the tail chain)
                    eng = nc.vector if last_head else nc.gpsimd
                    eng.tensor_copy(ahi[qrows, pair, qts],
                                    anorm[qrows, pair, qts])
                    eng.tensor_tensor(
                        alo[qrows, pair, qts],
                        anorm[qrows, pair, qts],
                        ahi[qrows, pair, qts],
                        SUB)

                # ---- master schedule: kv0/q00 up front with Activation-
                # assisted copies (exp idle), V-transpose 0 borrows the psA
                # ring; later h tiles are DMA-gated so they pump as filler.
                for wi in range(9):
                    wps = psA.tile([128, TT], F32, tag="att",
                                   name=f"warmmm{wi}")
                    nc.tensor.matmul(wps[:, 0:256], wa[:, 0:128],
                                     wa[:, 0:256], start=True, stop=True)
                proj_kv(0, slot=(psP, "proj"), act_copy=True)
                proj_q(0, 0, slot=(psP, "proj"), act_copy=True)
                transp(0, slot=(psA, "att"))
                proj_kv(1)
                filler.append(lambda: proj_kv(2))
                filler.append(lambda: transp(1))
                filler.append(lambda: proj_q(1, 0, slot=(psA, "att")))
                filler.append(lambda: proj_kv(3))
                filler.append(lambda: transp(2))
                filler.append(lambda: transp(3))
                for tt in range(1, NTT):
                    for rc in range(2):
                        filler.append(
                            lambda rc=rc, tt=tt: proj_q(rc, tt))

                heads = [(pair, h2, qt)
                         for qt in range(NTT)
                         for pair in range(2)
                         for h2 in range(2)]

                def head_done(i):
                    if i % 4 == 3:
                        qt = heads[i][2]
                        last = i == len(heads) - 1
                        for tch in range(4 * qt, 4 * qt + 4):
                            for ht in range(2):
                                filler.append(
                                    lambda tch=tch, ht=ht, last=last:
                                    outproj_tile(tch, ht, last=last))

                nxt = 0

                def start_next():
                    nonlocal nxt
                    if nxt >= len(heads):
                        return None
                    g = attention_head(*heads[nxt], offload=(nxt >= 4),
                                       last_head=(nxt == len(heads) - 1))
                    nxt += 1
                    return (nxt - 1, g)

                nproj_fill = len(filler)
                slots = [start_next(), None]
                stagger = 6
                step = 0
                while any(slots):
                    for si in range(2):
                        if slots[si] is None:
                            continue
                        i, g = slots[si]
                        try:
                            next(g)
                            # drain the projection fillers at double rate so
                            # their PSUM->rope chains stay ahead of attention
                            pump(2 if step < nproj_fill else 1)
                            step += 1
                            if stagger is not None:
                                stagger -= 1
                                if stagger == 0:
                                    slots[1] = start_next()
                                    stagger = None
                        except StopIteration:
                            head_done(i)
                            slots[si] = start_next()
                while filler:
                    pump(1)
    nc.finalize()
    return nc


def _get_nc():
    global _nc_cache
    if _nc_cache is None:
        _nc_cache = _build_bass()
    return _nc_cache


def _hilo(x):
    hi = x.astype(_E4)
    lo = (x - hi.astype(np.float32)).astype(_E5)
    return hi, lo


def _shard_inputs(hidden_states, cos, sin, w_qkv, w_o):
    """Build per-core input maps. Core c = (b = c // 4, g = c % 4)."""
    cosT = cos.T.astype(np.float32)                                # [64, S]
    sinT = sin.T.astype(np.float32)
    sinmod = np.concatenate([-sinT[0:32], sinT[32:64]], axis=0)
    cosc = np.ascontiguousarray(cosT / 16.0).astype(_BF16)
    sinc = np.ascontiguousarray(sinmod / 16.0).astype(_BF16)

    # h packed [p, j, i, t]: feature 256j + 128i + p
    hsplit = []
    for b in range(B):
        ht = hidden_states[b].T.astype(np.float32)                 # [1024, S]
        hp = np.ascontiguousarray(
            ht.reshape(4, 2, 128, S).transpose(2, 0, 1, 3))        # [128,4,2,S]
        hsplit.append(_hilo(hp))
    in_maps = []
    for c in range(NCORES):
        b, g = divmod(c, 4)
        q_rows = w_qkv[256 * g: 256 * g + 256]
        k_rows = w_qkv[1024 + 64 * g: 1024 + 64 * g + 64]
        v_rows = w_qkv[1280 + 64 * g: 1280 + 64 * g + 64]
        wqk = np.concatenate([q_rows, k_rows, v_rows], axis=0)     # [384, 1024]
        # x16 into fp8 range; [p, rc, j, i, m] with h = 256j+128i+p
        wqkT = (wqk.T * 16.0).astype(np.float32)                   # [1024, 384]
        wpk = np.ascontiguousarray(
            wqkT.reshape(4, 2, 128, 3, 128).transpose(2, 3, 0, 1, 4))
        whi_a, wlo_a = _hilo(wpk)
        woTf = (w_o[:, 256 * g: 256 * g + 256].T * 16.0).astype(np.float32)
        wo_pk = np.ascontiguousarray(
            woTf.reshape(2, 128, HID).transpose(1, 0, 2))          # [128,2,HID]
        wohi_a, wolo_a = _hilo(wo_pk)
        in_maps.append(
            {
                "hhi": hsplit[b][0],
                "hlo": hsplit[b][1],
                "whi": whi_a,
                "wlo": wlo_a,
                "wohi": wohi_a,
                "wolo": wolo_a,
                "cosd": cosc,
                "sind": sinc,
            }
        )
    return in_maps


def _run(inputs, **spmd_kwargs):
    from concourse.bass_utils import run_bass_kernel_spmd

    nc = _get_nc()
    in_maps = _shard_inputs(**inputs)
    res = run_bass_kernel_spmd(
        nc, in_maps, core_ids=list(range(NCORES)), **spmd_kwargs
    )
    outs = []
    for b in range(B):
        acc = res.results[4 * b]["out"].astype(np.float32)
        for g in range(1, 4):
            acc = acc + res.results[4 * b + g]["out"].astype(np.float32)
        outs.append(acc * OUT_SCALE)
    return np.stack(outs, axis=0), res


def kernel(**inputs):
    out, _ = _run(inputs)
    return out


# revision 12
# speedup vs baseline: 1.4109x; 1.0254x over previous
"""GQA attention layer on 8 Trainium2 NeuronCores — v6.

v5 + fp8 hi/lo DoubleRow for both projections:
- QKV proj: host decomposes h and w_qkv into e4m3 hi + e5m2 lo; each K=256
  block is 3 DoubleRow matmuls (hi*hi + hi*lo + lo*hi) at half the PE cost
  of bf16, with ~bf16 accuracy (hi+lo carries ~14 mantissa bits).
- out proj: Pool computes anorm hi/lo from the bf16 normalize result;
  w_o decomposed on host. 3 DR matmuls replace 2 bf16 ones per tile.
Scaling: weights x16 into fp8 range, q rope tables fold SCALE/16, k tables
fold 1/16, V folds 1/16 at the vT copy, vaug ones-column = 1/32 so
anorm = 32*attn (fp8-friendly), w_o x16 -> output is 512x; host divides.
"""

import collections

import numpy as np
import ml_dtypes

B, S, HID = 2, 2048, 1024
NH, NKV, D = 16, 4, 64
SCALE = float(D ** -0.5)
NCORES = 8
TT = 512          # token tile
NTT = S // TT     # 4
KC = S // 128     # 16 key chunks
NCP = KC // 2     # 8 chunk pairs
OUT_SCALE = 1.0 / 512.0

_BF16 = ml_dtypes.bfloat16
_E4 = ml_dtypes.float8_e4m3
_E5 = ml_dtypes.float8_e5m2

_nc_cache = None


def _build_bass():
    import concourse.bass as bass
    import concourse.mybir as mybir
    import concourse.tile as tile
    from concourse import bacc
    from concourse.masks import make_identity

    BF = mybir.dt.bfloat16
    F32 = mybir.dt.float32
    I16 = mybir.dt.int16
    E4 = mybir.dt.float8e4
    E5 = mybir.dt.float8e5
    AF = mybir.ActivationFunctionType
    MULT = mybir.AluOpType.mult
    ADD = mybir.AluOpType.add
    SUB = mybir.AluOpType.subtract
    DR = mybir.MatmulPerfMode.DoubleRow
    # Schraudolph exp for offloaded tiles: bitcast(int16(s*A + B)) ~ exp(s)
    SCH_A = 184.66496280558537 * SCALE   # 128/ln2, scores carry 1/SCALE
    SCH_B = 16256.0 - 5.75 + 0.5         # bias center + truncation fix

    nc = bacc.Bacc()
    # h split hi/lo, packed [p, j, i, t]: h feature 256j + 128i + p
    hhi = nc.dram_tensor("hhi", (128, 4, 2, S), E4, kind="ExternalInput")
    hlo = nc.dram_tensor("hlo", (128, 4, 2, S), E5, kind="ExternalInput")
    # wqk split hi/lo, packed [p, rc, j, i, m]
    whi = nc.dram_tensor("whi", (128, 3, 4, 2, 128), E4, kind="ExternalInput")
    wlo = nc.dram_tensor("wlo", (128, 3, 4, 2, 128), E5, kind="ExternalInput")
    wohi = nc.dram_tensor("wohi", (128, 2, HID), E4, kind="ExternalInput")
    wolo = nc.dram_tensor("wolo", (128, 2, HID), E5, kind="ExternalInput")
    # shared q/k rope tables (/16); SCALE is applied by the exp activation
    cosd = nc.dram_tensor("cosd", (64, S), BF, kind="ExternalInput")
    sind = nc.dram_tensor("sind", (64, S), BF, kind="ExternalInput")
    out = nc.dram_tensor("out", (S, HID), BF, kind="ExternalOutput")

    with tile.TileContext(nc) as tc:
        with (
            tc.tile_pool(name="persist", bufs=1) as pp,
            tc.tile_pool(name="pbfp", bufs=3) as pbfp,
            tc.tile_pool(name="rope", bufs=3) as rp,
            tc.tile_pool(name="exps", bufs=6) as ep,
            tc.tile_pool(name="norm", bufs=4) as np_,
            tc.tile_pool(name="outsb", bufs=4) as op_,
        ):
            # ---- persistent SBUF tiles + input loads, kv-first order
            hhi_sb = pp.tile([128, 4, 2, S], E4, tag="hhi_sb")
            hlo_sb = pp.tile([128, 4, 2, S], E5, tag="hlo_sb")
            whi_sb = pp.tile([128, 3, 4, 2, 128], E4, tag="whi_sb")
            wlo_sb = pp.tile([128, 3, 4, 2, 128], E5, tag="wlo_sb")
            wohi_sb = pp.tile([128, 2, HID], E4, tag="wohi_sb")
            wolo_sb = pp.tile([128, 2, HID], E5, tag="wolo_sb")
            # [128, S]: rows 0:64 DMA'd, rows 64:128 duplicated on-device
            cos_sb = pp.tile([128, S], BF, tag="cos_sb")
            sin_sb = pp.tile([128, S], BF, tag="sin_sb")

            def h_slice(t0, t1):
                for hd, hs in ((hhi, hhi_sb), (hlo, hlo_sb)):
                    nc.sync.dma_start(hs[:, :, :, t0:t1], hd[:, :, :, t0:t1])

            nc.sync.dma_start(whi_sb[:, 2], whi[:, 2])
            nc.sync.dma_start(hhi_sb[:, 0:2, :, 0:TT], hhi[:, 0:2, :, 0:TT])
            nc.sync.dma_start(hhi_sb[:, 2:4, :, 0:TT], hhi[:, 2:4, :, 0:TT])
            nc.sync.dma_start(whi_sb[:, 0:2], whi[:, 0:2])
            nc.sync.dma_start(wlo_sb[:, 2], wlo[:, 2])
            nc.sync.dma_start(hlo_sb[:, :, :, 0:TT], hlo[:, :, :, 0:TT])
            nc.sync.dma_start(wlo_sb[:, 0:2], wlo[:, 0:2])
            for tt in range(NTT):
                tts_ = bass.ts(tt, TT)
                if tt > 0:
                    h_slice(tt * TT, (tt + 1) * TT)
                nc.sync.dma_start(cos_sb[0:64, tts_], cosd[:, tts_])
                nc.sync.dma_start(sin_sb[0:64, tts_], sind[:, tts_])
                # Pool duplicates the tables onto partitions 64:128
                nc.gpsimd.tensor_copy(cos_sb[64:128, tts_],
                                      cos_sb[0:64, tts_])
                nc.gpsimd.tensor_copy(sin_sb[64:128, tts_],
                                      sin_sb[0:64, tts_])
            nc.sync.dma_start(wohi_sb[:], wohi[:])
            nc.sync.dma_start(wolo_sb[:], wolo[:])

            ident = pp.tile([64, 64], BF, tag="ident")
            make_identity(nc, ident[:])
            warm = pp.tile([1, 8], F32, tag="warm")
            nc.any.memset(warm[:], 0.0)
            nc.scalar.activation(warm[:], warm[:], AF.Exp)
            # keep the tensor engine busy on junk matmuls while the first
            # h/w DMAs land, so the p-state ramp completes before real work
            wa = pp.tile([128, TT], BF, tag="wa")
            nc.gpsimd.memset(wa[:], 0.5)

            qrot = [pp.tile([128, S], BF, tag=f"qrot{p}", name=f"qrot{p}")
                    for p in range(2)]
            k2 = pp.tile([128, S], BF, tag="k2")
            vT = pp.tile([64, S], BF, tag="vT")
            vaug = pp.tile([128, KC, 65], BF, tag="vaug")
            nc.any.memset(vaug[:], 1.0 / 32.0)
            # anorm = 32*attn: bf16 full + fp8 hi/lo for the DR outproj,
            # packed [p, oc(=pair), t]
            anorm = pp.tile([128, 2, S], BF, tag="anorm")
            ahi = pp.tile([128, 2, S], E4, tag="ahi")
            alo = pp.tile([128, 2, S], E5, tag="alo")

            with (
                tc.tile_pool(name="psP", bufs=2, space="PSUM") as psP,
                tc.tile_pool(name="psS", bufs=2, space="PSUM") as psS,
                tc.tile_pool(name="psA", bufs=2, space="PSUM") as psA,
            ):

                def rope(pbf, dest, rows, tts, tag):
                    """Pool builds the 32-block-swapped copy; DVE runs
                    same-partition bf16 2x-mode multiply/add ops against the
                    compact shared [64, S] tables."""
                    sg = rp.tile([128, TT], BF, tag=f"sg{tag}")
                    for blk in range(rows // 32):
                        src = blk ^ 1
                        nc.gpsimd.tensor_copy(
                            sg[32 * blk: 32 * blk + 32, :],
                            pbf[32 * src: 32 * src + 32, :])
                    t1 = rp.tile([128, TT], BF, tag=f"t1{tag}")
                    rt = rp.tile([128, TT], BF, tag=f"rt{tag}")
                    nc.vector.tensor_tensor(
                        t1[0:rows, :], pbf[0:rows, :], cos_sb[0:rows, tts],
                        MULT)
                    nc.vector.tensor_tensor(
                        rt[0:rows, :], sg[0:rows, :], sin_sb[0:rows, tts],
                        MULT)
                    nc.vector.tensor_tensor(
                        dest, t1[0:rows, :], rt[0:rows, :], ADD)

                def proj(rc, tt, name, slot):
                    """hi/lo DoubleRow projection: 12 accumulating DR matmuls
                    (4 K=256 blocks x {hi*hi, hi*lo, lo*hi}). slot borrows an
                    idle PSUM ring early in the prologue."""
                    tts = bass.ts(tt, TT)
                    pool, tag = slot
                    if tag == "sc":
                        ps = pool.tile([128, 2 * TT], F32, tag="sc",
                                       name=name)[:, 0:TT]
                    else:
                        ps = pool.tile([128, TT], F32, tag=tag, name=name)
                    steps = []
                    for j in range(4):
                        steps.append((whi_sb[:, rc, j], hhi_sb[:, j, :, tts]))
                    for j in range(4):
                        steps.append((whi_sb[:, rc, j], hlo_sb[:, j, :, tts]))
                        steps.append((wlo_sb[:, rc, j], hhi_sb[:, j, :, tts]))
                    for si, (w, x) in enumerate(steps):
                        nc.tensor.matmul(
                            ps[:], w, x,
                            start=(si == 0), stop=(si == len(steps) - 1),
                            perf_mode=DR)
                    return ps, tts

                def proj_kv(tt, slot=(psP, "proj"), act_copy=False):
                    ps, tts = proj(2, tt, f"projkv_{tt}", slot)
                    kbf = pbfp.tile([128, TT], BF, tag="pbf", name=f"kbf{tt}")
                    if act_copy:
                        nc.scalar.copy(kbf[0:64, :], ps[0:64, :])
                        nc.scalar.copy(vT[:, tts], ps[64:128, :])
                        nc.vector.tensor_scalar_mul(vT[:, tts], vT[:, tts],
                                                    1.0 / 16.0)
                    else:
                        nc.vector.tensor_copy(kbf[0:64, :], ps[0:64, :])
                        # v = ps/16
                        nc.vector.tensor_scalar_mul(vT[:, tts], ps[64:128, :],
                                                    1.0 / 16.0)
                    rope(kbf, k2[0:64, tts], 64, tts, "k")
                    nc.gpsimd.tensor_copy(k2[64:128, tts], k2[0:64, tts])

                def transp(tt, slot=(psP, "proj")):
                    pool, tag = slot
                    pt = pool.tile([128, 4, 64], BF, tag=tag,
                                   name=f"vt{tt}")
                    for ci in range(4):
                        c = 4 * tt + ci
                        nc.tensor.transpose(pt[:, ci, :],
                                            vT[:, bass.ts(c, 128)], ident[:])
                    nc.vector.tensor_copy(vaug[:, 4 * tt: 4 * tt + 4, 0:64],
                                          pt[:])

                def proj_q(rc, tt, slot=(psP, "proj"), act_copy=False):
                    ps, tts = proj(rc, tt, f"projq{rc}_{tt}", slot)
                    pbf = pbfp.tile([128, TT], BF, tag="pbf",
                                    name=f"qbf{rc}_{tt}")
                    nc.vector.tensor_copy(pbf[:], ps[:])
                    rope(pbf, qrot[rc][:, tts], 128, tts, "q")

                def outproj_tile(tch, ht, last=False):
                    tcs = bass.ts(tch, 128)
                    hts = bass.ts(ht, TT)
                    po = psP.tile([128, TT], F32, tag="proj",
                                  name=f"po{tch}_{ht}")
                    terms = [(ahi[:, :, tcs], wohi_sb[:, :, hts]),
                             (ahi[:, :, tcs], wolo_sb[:, :, hts]),
                             (alo[:, :, tcs], wohi_sb[:, :, hts])]
                    for si, (a, w) in enumerate(terms):
                        nc.tensor.matmul(
                            po[:], a, w,
                            start=(si == 0), stop=(si == len(terms) - 1),
                            perf_mode=DR)
                    ob = op_.tile([128, TT], BF, tag="ob")
                    if (tch + ht) % 2 == 0:
                        nc.scalar.copy(ob[:], po[:])
                    else:
                        nc.vector.tensor_copy(ob[:], po[:])
                    nc.sync.dma_start(out[tcs, hts], ob[:])

                # ---- filler queue
                filler = collections.deque()

                def pump(n=1):
                    for _ in range(n):
                        if not filler:
                            return
                        filler.popleft()()

                def attention_head(pair, h2, qt, offload=True,
                                   last_head=False):
                    qts = bass.ts(qt, TT)
                    qrows = slice(64 * h2, 64 * h2 + 64)
                    pacc = psA.tile([65, TT], F32, tag="att",
                                    name=f"att{pair}_{h2}_{qt}")
                    pending = collections.deque()

                    def drain_pending(keep):
                        while len(pending) > keep:
                            pex, pcp = pending.popleft()
                            for j in range(2):
                                c = 2 * pcp + j
                                nc.tensor.matmul(
                                    pacc[:], vaug[:, c, :],
                                    pex[:, 512 * j: 512 * j + 512],
                                    start=(c == 0), stop=(c == KC - 1))

                    for cp in range(NCP):
                        sc = psS.tile([128, 2 * TT], F32, tag="sc",
                                      name=f"sc{pair}_{h2}_{qt}_{cp}")
                        for j in range(2):
                            c = 2 * cp + j
                            nc.tensor.matmul(
                                sc[:, bass.ts(j, TT)],
                                k2[qrows, bass.ts(c, 128)],
                                qrot[pair][qrows, qts],
                                start=True, stop=True,
                                tile_position=(64 * h2, 0))
                        if cp == 3 and offload:
                            # offload this tile's exp to DVE (Schraudolph);
                            # the ~2% approx error on 1/8 of the keys is
                            # within budget and relieves the pacing engine
                            exi = ep.tile([128, 2 * TT], I16, tag="exps")
                            nc.vector.tensor_scalar(exi[:], sc[:],
                                                    SCH_A, SCH_B, MULT, ADD)
                            ex = exi[:].bitcast(BF)
                        else:
                            ext = ep.tile([128, 2 * TT], BF, tag="exp")
                            nc.scalar.activation(ext[:], sc[:], AF.Exp,
                                                 scale=SCALE)
                            ex = ext[:]
                        pending.append((ex, cp))
                        # attnV trails scores by two cpairs so the exp
                        # semaphore has always fired by the time the PE
                        # reaches the accumulation matmuls
                        drain_pending(2)
                        yield
                    drain_pending(0)
                    # copy pacc to SBUF bf16 immediately -> psA bank freed in
                    # one op; normalize runs SBUF-side in bf16 2x-mode ops
                    att = np_.tile([64, TT], F32, tag="att_sb")
                    nc.vector.tensor_copy(att[:], pacc[0:64, :])
                    # den staged to a partition-0 tile: reciprocal reading an
                    # SBUF slice at base partition 64 returns garbage on HW
                    den = np_.tile([1, TT], F32, tag="den")
                    nc.vector.tensor_copy(den[:], pacc[64:65, :])
                    rec = np_.tile([1, TT], F32, tag="rec")
                    nc.vector.reciprocal_approx_fast(rec[:], den[:])
                    bc = np_.tile([64, TT], F32, tag="bc")
                    nc.gpsimd.partition_broadcast(bc[:], rec[:])
                    nc.vector.tensor_tensor(
                        anorm[qrows, pair, qts], att[0:64, :], bc[:], MULT)
                    # fp8 hi/lo for the DR outproj (Pool; DVE for the last
                    # head to shorten the tail chain)
                    eng = nc.vector if last_head else nc.gpsimd
                    eng.tensor_copy(ahi[qrows, pair, qts],
                                    anorm[qrows, pair, qts])
                    eng.tensor_tensor(
                        alo[qrows, pair, qts],
                        anorm[qrows, pair, qts],
                        ahi[qrows, pair, qts],
                        SUB)

                # ---- master schedule: kv0/q00 up front with Activation-
                # assisted copies (exp idle), V-transpose 0 borrows the psA
                # ring; later h tiles are DMA-gated so they pump as filler.
                for wi in range(9):
                    wps = psA.tile([128, TT], F32, tag="att",
                                   name=f"warmmm{wi}")
                    nc.tensor.matmul(wps[:, 0:256], wa[:, 0:128],
                                     wa[:, 0:256], start=True, stop=True)
                proj_kv(0, slot=(psP, "proj"), act_copy=True)
                proj_q(0, 0, slot=(psP, "proj"), act_copy=True)
                transp(0, slot=(psA, "att"))
                proj_kv(1)
                filler.append(lambda: proj_kv(2))
                filler.append(lambda: transp(1))
                filler.append(lambda: proj_q(1, 0, slot=(psA, "att")))
                filler.append(lambda: proj_kv(3))
                filler.append(lambda: transp(2))
                filler.append(lambda: transp(3))
                for tt in range(1, NTT):
                    for rc in range(2):
                        filler.append(
                            lambda rc=rc, tt=tt: proj_q(rc, tt))

                heads = [(pair, h2, qt)
                         for qt in range(NTT)
                         for pair in range(2)
                         for h2 in range(2)]

                def head_done(i):
                    if i % 4 == 3:
                        qt = heads[i][2]
                        last = i == len(heads) - 1
                        for tch in range(4 * qt, 4 * qt + 4):
                            for ht in range(2):
                                filler.append(
                                    lambda tch=tch, ht=ht, last=last:
                                    outproj_tile(tch, ht, last=last))

                nxt = 0

                def start_next():
                    nonlocal nxt
                    if nxt >= len(heads):
                        return None
                    g = attention_head(*heads[nxt], offload=(nxt >= 4),
                                       last_head=(nxt == len(heads) - 1))
                    nxt += 1
                    return (nxt - 1, g)

                nproj_fill = len(filler)
                slots = [start_next(), None]
                stagger = 6
                step = 0
                while any(slots):
                    for si in range(2):
                        if slots[si] is None:
                            continue
                        i, g = slots[si]
                        try:
                            next(g)
                            # projection fillers drain at double rate (their
                            # PSUM->rope chains must stay ahead); outproj
                            # fillers at half rate so they cover the whole
                            # q-tile's rounds instead of bunching
                            if step < nproj_fill:
                                pump(2)
                            elif step % 2 == 0:
                                pump(1)
                            step += 1
                            if stagger is not None:
                                stagger -= 1
                                if stagger == 0:
                                    slots[1] = start_next()
                                    stagger = None
                        except StopIteration:
                            head_done(i)
                            slots[si] = start_next()
                while filler:
                    pump(1)
    nc.finalize()
    return nc


def _get_nc():
    global _nc_cache
    if _nc_cache is None:
        _nc_cache = _build_bass()
    return _nc_cache


def _hilo(x):
    hi = x.astype(_E4)
    lo = (x - hi.astype(np.float32)).astype(_E5)
    return hi, lo


def _shard_inputs(hidden_states, cos, sin, w_qkv, w_o):
    """Build per-core input maps. Core c = (b = c // 4, g = c % 4)."""
    cosT = cos.T.astype(np.float32)                                # [64, S]
    sinT = sin.T.astype(np.float32)
    sinmod = np.concatenate([-sinT[0:32], sinT[32:64]], axis=0)
    cosc = np.ascontiguousarray(cosT / 16.0).astype(_BF16)
    sinc = np.ascontiguousarray(sinmod / 16.0).astype(_BF16)

    # h packed [p, j, i, t]: feature 256j + 128i + p
    hsplit = []
    for b in range(B):
        ht = hidden_states[b].T.astype(np.float32)                 # [1024, S]
        hp = np.ascontiguousarray(
            ht.reshape(4, 2, 128, S).transpose(2, 0, 1, 3))        # [128,4,2,S]
        hsplit.append(_hilo(hp))
    in_maps = []
    for c in range(NCORES):
        b, g = divmod(c, 4)
        q_rows = w_qkv[256 * g: 256 * g + 256]
        k_rows = w_qkv[1024 + 64 * g: 1024 + 64 * g + 64]
        v_rows = w_qkv[1280 + 64 * g: 1280 + 64 * g + 64]
        wqk = np.concatenate([q_rows, k_rows, v_rows], axis=0)     # [384, 1024]
        # x16 into fp8 range; [p, rc, j, i, m] with h = 256j+128i+p
        wqkT = (wqk.T * 16.0).astype(np.float32)                   # [1024, 384]
        wpk = np.ascontiguousarray(
            wqkT.reshape(4, 2, 128, 3, 128).transpose(2, 3, 0, 1, 4))
        whi_a, wlo_a = _hilo(wpk)
        woTf = (w_o[:, 256 * g: 256 * g + 256].T * 16.0).astype(np.float32)
        wo_pk = np.ascontiguousarray(
            woTf.reshape(2, 128, HID).transpose(1, 0, 2))          # [128,2,HID]
        wohi_a, wolo_a = _hilo(wo_pk)
        in_maps.append(
            {
                "hhi": hsplit[b][0],
                "hlo": hsplit[b][1],
                "whi": whi_a,
                "wlo": wlo_a,
                "wohi": wohi_a,
                "wolo": wolo_a,
                "cosd": cosc,
                "sind": sinc,
            }
        )
    return in_maps


def _run(inputs, **spmd_kwargs):
    from concourse.bass_utils import run_bass_kernel_spmd

    nc = _get_nc()
    in_maps = _shard_inputs(**inputs)
    res = run_bass_kernel_spmd(
        nc, in_maps, core_ids=list(range(NCORES)), **spmd_kwargs
    )
    outs = []
    for b in range(B):
        acc = res.results[4 * b]["out"].astype(np.float32)
        for g in range(1, 4):
            acc = acc + res.results[4 * b + g]["out"].astype(np.float32)
        outs.append(acc * OUT_SCALE)
    return np.stack(outs, axis=0), res


def kernel(**inputs):
    out, _ = _run(inputs)
    return out


# revision 13
# speedup vs baseline: 1.4183x; 1.0052x over previous
"""GQA attention layer on 8 Trainium2 NeuronCores — v6.

v5 + fp8 hi/lo DoubleRow for both projections:
- QKV proj: host decomposes h and w_qkv into e4m3 hi + e5m2 lo; each K=256
  block is 3 DoubleRow matmuls (hi*hi + hi*lo + lo*hi) at half the PE cost
  of bf16, with ~bf16 accuracy (hi+lo carries ~14 mantissa bits).
- out proj: Pool computes anorm hi/lo from the bf16 normalize result;
  w_o decomposed on host. 3 DR matmuls replace 2 bf16 ones per tile.
Scaling: weights x16 into fp8 range, q rope tables fold SCALE/16, k tables
fold 1/16, V folds 1/16 at the vT copy, vaug ones-column = 1/32 so
anorm = 32*attn (fp8-friendly), w_o x16 -> output is 512x; host divides.
"""

import collections

import numpy as np
import ml_dtypes

B, S, HID = 2, 2048, 1024
NH, NKV, D = 16, 4, 64
SCALE = float(D ** -0.5)
NCORES = 8
TT = 512          # token tile
NTT = S // TT     # 4
KC = S // 128     # 16 key chunks
NCP = KC // 2     # 8 chunk pairs
OUT_SCALE = 1.0 / 512.0

_BF16 = ml_dtypes.bfloat16
_E4 = ml_dtypes.float8_e4m3
_E5 = ml_dtypes.float8_e5m2

_nc_cache = None


def _build_bass():
    import concourse.bass as bass
    import concourse.mybir as mybir
    import concourse.tile as tile
    from concourse import bacc
    from concourse.masks import make_identity

    BF = mybir.dt.bfloat16
    F32 = mybir.dt.float32
    I16 = mybir.dt.int16
    E4 = mybir.dt.float8e4
    E5 = mybir.dt.float8e5
    AF = mybir.ActivationFunctionType
    MULT = mybir.AluOpType.mult
    ADD = mybir.AluOpType.add
    SUB = mybir.AluOpType.subtract
    DR = mybir.MatmulPerfMode.DoubleRow
    # Schraudolph exp for offloaded tiles: bitcast(int16(s*A + B)) ~ exp(s)
    SCH_A = 184.66496280558537 * SCALE   # 128/ln2, scores carry 1/SCALE
    SCH_B = 16256.0 - 5.75 + 0.5         # bias center + truncation fix

    nc = bacc.Bacc()
    # h split hi/lo, packed [p, j, i, t]: h feature 256j + 128i + p
    hhi = nc.dram_tensor("hhi", (128, 4, 2, S), E4, kind="ExternalInput")
    hlo = nc.dram_tensor("hlo", (128, 4, 2, S), E5, kind="ExternalInput")
    # wqk split hi/lo, packed [p, rc, j, i, m]
    whi = nc.dram_tensor("whi", (128, 3, 4, 2, 128), E4, kind="ExternalInput")
    wlo = nc.dram_tensor("wlo", (128, 3, 4, 2, 128), E5, kind="ExternalInput")
    wohi = nc.dram_tensor("wohi", (128, 2, HID), E4, kind="ExternalInput")
    wolo = nc.dram_tensor("wolo", (128, 2, HID), E5, kind="ExternalInput")
    # shared q/k rope tables (/16); SCALE is applied by the exp activation
    cosd = nc.dram_tensor("cosd", (64, S), BF, kind="ExternalInput")
    sind = nc.dram_tensor("sind", (64, S), BF, kind="ExternalInput")
    out = nc.dram_tensor("out", (S, HID), BF, kind="ExternalOutput")

    with tile.TileContext(nc) as tc:
        with (
            tc.tile_pool(name="persist", bufs=1) as pp,
            tc.tile_pool(name="pbfp", bufs=3) as pbfp,
            tc.tile_pool(name="rope", bufs=3) as rp,
            tc.tile_pool(name="exps", bufs=6) as ep,
            tc.tile_pool(name="norm", bufs=4) as np_,
            tc.tile_pool(name="outsb", bufs=4) as op_,
        ):
            # ---- persistent SBUF tiles + input loads, kv-first order
            hhi_sb = pp.tile([128, 4, 2, S], E4, tag="hhi_sb")
            hlo_sb = pp.tile([128, 4, 2, S], E5, tag="hlo_sb")
            whi_sb = pp.tile([128, 3, 4, 2, 128], E4, tag="whi_sb")
            wlo_sb = pp.tile([128, 3, 4, 2, 128], E5, tag="wlo_sb")
            wohi_sb = pp.tile([128, 2, HID], E4, tag="wohi_sb")
            wolo_sb = pp.tile([128, 2, HID], E5, tag="wolo_sb")
            # [128, S]: rows 0:64 DMA'd, rows 64:128 duplicated on-device
            cos_sb = pp.tile([128, S], BF, tag="cos_sb")
            sin_sb = pp.tile([128, S], BF, tag="sin_sb")

            def h_slice(t0, t1):
                for hd, hs in ((hhi, hhi_sb), (hlo, hlo_sb)):
                    nc.sync.dma_start(hs[:, :, :, t0:t1], hd[:, :, :, t0:t1])

            nc.sync.dma_start(whi_sb[:, 2], whi[:, 2])
            nc.sync.dma_start(hhi_sb[:, 0:2, :, 0:TT], hhi[:, 0:2, :, 0:TT])
            nc.sync.dma_start(hhi_sb[:, 2:4, :, 0:TT], hhi[:, 2:4, :, 0:TT])
            nc.sync.dma_start(whi_sb[:, 0:2], whi[:, 0:2])
            nc.sync.dma_start(wlo_sb[:, 2], wlo[:, 2])
            nc.sync.dma_start(hlo_sb[:, :, :, 0:TT], hlo[:, :, :, 0:TT])
            nc.sync.dma_start(wlo_sb[:, 0:2], wlo[:, 0:2])
            for tt in range(NTT):
                tts_ = bass.ts(tt, TT)
                if tt > 0:
                    h_slice(tt * TT, (tt + 1) * TT)
                nc.sync.dma_start(cos_sb[0:64, tts_], cosd[:, tts_])
                nc.sync.dma_start(sin_sb[0:64, tts_], sind[:, tts_])
                # Pool duplicates the tables onto partitions 64:128
                nc.gpsimd.tensor_copy(cos_sb[64:128, tts_],
                                      cos_sb[0:64, tts_])
                nc.gpsimd.tensor_copy(sin_sb[64:128, tts_],
                                      sin_sb[0:64, tts_])
            nc.sync.dma_start(wohi_sb[:], wohi[:])
            nc.sync.dma_start(wolo_sb[:], wolo[:])

            ident = pp.tile([64, 64], BF, tag="ident")
            make_identity(nc, ident[:])
            warm = pp.tile([1, 8], F32, tag="warm")
            nc.any.memset(warm[:], 0.0)
            nc.scalar.activation(warm[:], warm[:], AF.Exp)
            # keep the tensor engine busy on junk matmuls while the first
            # h/w DMAs land, so the p-state ramp completes before real work
            wa = pp.tile([128, TT], BF, tag="wa")
            nc.gpsimd.memset(wa[:], 0.5)

            qrot = [pp.tile([128, S], BF, tag=f"qrot{p}", name=f"qrot{p}")
                    for p in range(2)]
            k2 = pp.tile([128, S], BF, tag="k2")
            vT = pp.tile([64, S], BF, tag="vT")
            vaug = pp.tile([128, KC, 65], BF, tag="vaug")
            nc.any.memset(vaug[:], 1.0 / 32.0)
            # anorm = 32*attn: bf16 full + fp8 hi/lo for the DR outproj,
            # packed [p, oc(=pair), t]
            anorm = pp.tile([128, 2, S], BF, tag="anorm")
            ahi = pp.tile([128, 2, S], E4, tag="ahi")
            alo = pp.tile([128, 2, S], E5, tag="alo")

            with (
                tc.tile_pool(name="psP", bufs=2, space="PSUM") as psP,
                tc.tile_pool(name="psS", bufs=2, space="PSUM") as psS,
                tc.tile_pool(name="psA", bufs=2, space="PSUM") as psA,
            ):

                def rope(pbf, dest, rows, tts, tag):
                    """Pool builds the 32-block-swapped copy; DVE runs
                    same-partition bf16 2x-mode multiply/add ops against the
                    compact shared [64, S] tables."""
                    sg = rp.tile([128, TT], BF, tag=f"sg{tag}")
                    for blk in range(rows // 32):
                        src = blk ^ 1
                        nc.gpsimd.tensor_copy(
                            sg[32 * blk: 32 * blk + 32, :],
                            pbf[32 * src: 32 * src + 32, :])
                    t1 = rp.tile([128, TT], BF, tag=f"t1{tag}")
                    rt = rp.tile([128, TT], BF, tag=f"rt{tag}")
                    nc.vector.tensor_tensor(
                        t1[0:rows, :], pbf[0:rows, :], cos_sb[0:rows, tts],
                        MULT)
                    nc.vector.tensor_tensor(
                        rt[0:rows, :], sg[0:rows, :], sin_sb[0:rows, tts],
                        MULT)
                    nc.vector.tensor_tensor(
                        dest, t1[0:rows, :], rt[0:rows, :], ADD)

                def proj(rc, tt, name, slot):
                    """hi/lo DoubleRow projection: 12 accumulating DR matmuls
                    (4 K=256 blocks x {hi*hi, hi*lo, lo*hi}). slot borrows an
                    idle PSUM ring early in the prologue."""
                    tts = bass.ts(tt, TT)
                    pool, tag = slot
                    if tag == "sc":
                        ps = pool.tile([128, 2 * TT], F32, tag="sc",
                                       name=name)[:, 0:TT]
                    else:
                        ps = pool.tile([128, TT], F32, tag=tag, name=name)
                    steps = []
                    for j in range(4):
                        steps.append((whi_sb[:, rc, j], hhi_sb[:, j, :, tts]))
                    for j in range(4):
                        steps.append((whi_sb[:, rc, j], hlo_sb[:, j, :, tts]))
                        steps.append((wlo_sb[:, rc, j], hhi_sb[:, j, :, tts]))
                    for si, (w, x) in enumerate(steps):
                        nc.tensor.matmul(
                            ps[:], w, x,
                            start=(si == 0), stop=(si == len(steps) - 1),
                            perf_mode=DR)
                    return ps, tts

                def proj_kv(tt, slot=(psP, "proj"), act_copy=False):
                    ps, tts = proj(2, tt, f"projkv_{tt}", slot)
                    kbf = pbfp.tile([128, TT], BF, tag="pbf", name=f"kbf{tt}")
                    if act_copy:
                        nc.scalar.copy(kbf[0:64, :], ps[0:64, :])
                        nc.scalar.copy(vT[:, tts], ps[64:128, :])
                        nc.vector.tensor_scalar_mul(vT[:, tts], vT[:, tts],
                                                    1.0 / 16.0)
                    else:
                        nc.vector.tensor_copy(kbf[0:64, :], ps[0:64, :])
                        # v = ps/16
                        nc.vector.tensor_scalar_mul(vT[:, tts], ps[64:128, :],
                                                    1.0 / 16.0)
                    rope(kbf, k2[0:64, tts], 64, tts, "k")
                    nc.gpsimd.tensor_copy(k2[64:128, tts], k2[0:64, tts])

                def transp(tt, slot=(psP, "proj")):
                    pool, tag = slot
                    pt = pool.tile([128, 4, 64], BF, tag=tag,
                                   name=f"vt{tt}")
                    for ci in range(4):
                        c = 4 * tt + ci
                        nc.tensor.transpose(pt[:, ci, :],
                                            vT[:, bass.ts(c, 128)], ident[:])
                    nc.vector.tensor_copy(vaug[:, 4 * tt: 4 * tt + 4, 0:64],
                                          pt[:])

                def proj_q(rc, tt, slot=(psP, "proj"), act_copy=False):
                    ps, tts = proj(rc, tt, f"projq{rc}_{tt}", slot)
                    pbf = pbfp.tile([128, TT], BF, tag="pbf",
                                    name=f"qbf{rc}_{tt}")
                    nc.vector.tensor_copy(pbf[:], ps[:])
                    rope(pbf, qrot[rc][:, tts], 128, tts, "q")

                def outproj_tile(tch, ht, last=False):
                    tcs = bass.ts(tch, 128)
                    hts = bass.ts(ht, TT)
                    po = psP.tile([128, TT], F32, tag="proj",
                                  name=f"po{tch}_{ht}")
                    terms = [(ahi[:, :, tcs], wohi_sb[:, :, hts]),
                             (ahi[:, :, tcs], wolo_sb[:, :, hts]),
                             (alo[:, :, tcs], wohi_sb[:, :, hts])]
                    for si, (a, w) in enumerate(terms):
                        nc.tensor.matmul(
                            po[:], a, w,
                            start=(si == 0), stop=(si == len(terms) - 1),
                            perf_mode=DR)
                    ob = op_.tile([128, TT], BF, tag="ob")
                    if (tch + ht) % 2 == 0:
                        nc.scalar.copy(ob[:], po[:])
                    else:
                        nc.vector.tensor_copy(ob[:], po[:])
                    nc.sync.dma_start(out[tcs, hts], ob[:])

                # ---- filler queue
                filler = collections.deque()

                def pump(n=1):
                    for _ in range(n):
                        if not filler:
                            return
                        filler.popleft()()

                def attention_head(pair, h2, qt, offload=True,
                                   last_head=False):
                    qts = bass.ts(qt, TT)
                    qrows = slice(64 * h2, 64 * h2 + 64)
                    pacc = psA.tile([65, TT], F32, tag="att",
                                    name=f"att{pair}_{h2}_{qt}")
                    pending = collections.deque()

                    def drain_pending(keep):
                        while len(pending) > keep:
                            pex, pcp = pending.popleft()
                            for j in range(2):
                                c = 2 * pcp + j
                                nc.tensor.matmul(
                                    pacc[:], vaug[:, c, :],
                                    pex[:, 512 * j: 512 * j + 512],
                                    start=(c == 0), stop=(c == KC - 1))

                    for cp in range(NCP):
                        sc = psS.tile([128, 2 * TT], F32, tag="sc",
                                      name=f"sc{pair}_{h2}_{qt}_{cp}")
                        for j in range(2):
                            c = 2 * cp + j
                            nc.tensor.matmul(
                                sc[:, bass.ts(j, TT)],
                                k2[qrows, bass.ts(c, 128)],
                                qrot[pair][qrows, qts],
                                start=True, stop=True,
                                tile_position=(64 * h2, 0))
                        if cp == 3 and offload:
                            # offload this tile's exp to DVE (Schraudolph);
                            # the ~2% approx error on 1/8 of the keys is
                            # within budget and relieves the pacing engine
                            exi = ep.tile([128, 2 * TT], I16, tag="exps")
                            nc.vector.tensor_scalar(exi[:], sc[:],
                                                    SCH_A, SCH_B, MULT, ADD)
                            ex = exi[:].bitcast(BF)
                        else:
                            ext = ep.tile([128, 2 * TT], BF, tag="exp")
                            nc.scalar.activation(ext[:], sc[:], AF.Exp,
                                                 scale=SCALE)
                            ex = ext[:]
                        pending.append((ex, cp))
                        # attnV trails scores by two cpairs so the exp
                        # semaphore has always fired by the time the PE
                        # reaches the accumulation matmuls
                        drain_pending(2)
                        yield
                    drain_pending(0)
                    # copy pacc to SBUF bf16 immediately -> psA bank freed in
                    # one op; normalize runs SBUF-side in bf16 2x-mode ops
                    att = np_.tile([64, TT], F32, tag="att_sb")
                    nc.vector.tensor_copy(att[:], pacc[0:64, :])
                    # den staged to a partition-0 tile: reciprocal reading an
                    # SBUF slice at base partition 64 returns garbage on HW
                    den = np_.tile([1, TT], F32, tag="den")
                    nc.vector.tensor_copy(den[:], pacc[64:65, :])
                    rec = np_.tile([1, TT], F32, tag="rec")
                    nc.vector.reciprocal_approx_fast(rec[:], den[:])
                    bc = np_.tile([64, TT], F32, tag="bc")
                    nc.gpsimd.partition_broadcast(bc[:], rec[:])
                    nc.vector.tensor_tensor(
                        anorm[qrows, pair, qts], att[0:64, :], bc[:], MULT)
                    # fp8 hi/lo for the DR outproj (Pool; DVE for the last
                    # head to shorten the tail chain)
                    eng = nc.vector if last_head else nc.gpsimd
                    eng.tensor_copy(ahi[qrows, pair, qts],
                                    anorm[qrows, pair, qts])
                    eng.tensor_tensor(
                        alo[qrows, pair, qts],
                        anorm[qrows, pair, qts],
                        ahi[qrows, pair, qts],
                        SUB)

                # ---- master schedule: kv0/q00 up front with Activation-
                # assisted copies (exp idle), V-transpose 0 borrows the psA
                # ring; later h tiles are DMA-gated so they pump as filler.
                def warm_mms(n, label):
                    for wi in range(n):
                        wps = psA.tile([128, TT], F32, tag="att",
                                       name=f"warm{label}_{wi}")
                        nc.tensor.matmul(wps[:, 0:256], wa[:, 0:128],
                                         wa[:, 0:256], start=True, stop=True)

                warm_mms(16, "a")
                proj_kv(0, slot=(psP, "proj"), act_copy=True)
                proj_q(0, 0, slot=(psP, "proj"), act_copy=True)
                transp(0, slot=(psA, "att"))
                proj_kv(1)
                filler.append(lambda: proj_kv(2))
                filler.append(lambda: transp(1))
                filler.append(lambda: proj_q(1, 0, slot=(psA, "att")))
                filler.append(lambda: proj_kv(3))
                filler.append(lambda: transp(2))
                filler.append(lambda: transp(3))
                for tt in range(1, NTT):
                    for rc in range(2):
                        filler.append(
                            lambda rc=rc, tt=tt: proj_q(rc, tt))

                heads = [(pair, h2, qt)
                         for qt in range(NTT)
                         for pair in range(2)
                         for h2 in range(2)]

                def head_done(i):
                    if i % 4 == 3:
                        qt = heads[i][2]
                        last = i == len(heads) - 1
                        for tch in range(4 * qt, 4 * qt + 4):
                            for ht in range(2):
                                filler.append(
                                    lambda tch=tch, ht=ht, last=last:
                                    outproj_tile(tch, ht, last=last))

                nxt = 0

                def start_next():
                    nonlocal nxt
                    if nxt >= len(heads):
                        return None
                    g = attention_head(*heads[nxt], offload=(nxt >= 4),
                                       last_head=(nxt == len(heads) - 1))
                    nxt += 1
                    return (nxt - 1, g)

                nproj_fill = len(filler)
                slots = [start_next(), None]
                stagger = 6
                step = 0
                while any(slots):
                    for si in range(2):
                        if slots[si] is None:
                            continue
                        i, g = slots[si]
                        try:
                            next(g)
                            # projection fillers drain at double rate (their
                            # PSUM->rope chains must stay ahead); outproj
                            # fillers at half rate so they cover the whole
                            # q-tile's rounds instead of bunching
                            if step < nproj_fill:
                                pump(2)
                            elif step % 2 == 0:
                                pump(1)
                            step += 1
                            if stagger is not None:
                                stagger -= 1
                                if stagger == 0:
                                    slots[1] = start_next()
                                    stagger = None
                        except StopIteration:
                            head_done(i)
                            slots[si] = start_next()
                # bridge the last norm chain with junk matmuls so the
                # final outproj tiles run at full PE clock
                filler.appendleft(lambda: warm_mms(14, "t"))
                while filler:
                    pump(1)
    nc.finalize()
    return nc


def _get_nc():
    global _nc_cache
    if _nc_cache is None:
        _nc_cache = _build_bass()
    return _nc_cache


def _hilo(x):
    hi = x.astype(_E4)
    lo = (x - hi.astype(np.float32)).astype(_E5)
    return hi, lo


def _shard_inputs(hidden_states, cos, sin, w_qkv, w_o):
    """Build per-core input maps. Core c = (b = c // 4, g = c % 4)."""
    cosT = cos.T.astype(np.float32)                                # [64, S]
    sinT = sin.T.astype(np.float32)
    sinmod = np.concatenate([-sinT[0:32], sinT[32:64]], axis=0)
    cosc = np.ascontiguousarray(cosT / 16.0).astype(_BF16)
    sinc = np.ascontiguousarray(sinmod / 16.0).astype(_BF16)

    # h packed [p, j, i, t]: feature 256j + 128i + p
    hsplit = []
    for b in range(B):
        ht = hidden_states[b].T.astype(np.float32)                 # [1024, S]
        hp = np.ascontiguousarray(
            ht.reshape(4, 2, 128, S).transpose(2, 0, 1, 3))        # [128,4,2,S]
        hsplit.append(_hilo(hp))
    in_maps = []
    for c in range(NCORES):
        b, g = divmod(c, 4)
        q_rows = w_qkv[256 * g: 256 * g + 256]
        k_rows = w_qkv[1024 + 64 * g: 1024 + 64 * g + 64]
        v_rows = w_qkv[1280 + 64 * g: 1280 + 64 * g + 64]
        wqk = np.concatenate([q_rows, k_rows, v_rows], axis=0)     # [384, 1024]
        # x16 into fp8 range; [p, rc, j, i, m] with h = 256j+128i+p
        wqkT = (wqk.T * 16.0).astype(np.float32)                   # [1024, 384]
        wpk = np.ascontiguousarray(
            wqkT.reshape(4, 2, 128, 3, 128).transpose(2, 3, 0, 1, 4))
        whi_a, wlo_a = _hilo(wpk)
        woTf = (w_o[:, 256 * g: 256 * g + 256].T * 16.0).astype(np.float32)
        wo_pk = np.ascontiguousarray(
            woTf.reshape(2, 128, HID).transpose(1, 0, 2))          # [128,2,HID]
        wohi_a, wolo_a = _hilo(wo_pk)
        in_maps.append(
            {
                "hhi": hsplit[b][0],
                "hlo": hsplit[b][1],
                "whi": whi_a,
                "wlo": wlo_a,
                "wohi": wohi_a,
                "wolo": wolo_a,
                "cosd": cosc,
                "sind": sinc,
            }
        )
    return in_maps


def _run(inputs, **spmd_kwargs):
    from concourse.bass_utils import run_bass_kernel_spmd

    nc = _get_nc()
    in_maps = _shard_inputs(**inputs)
    res = run_bass_kernel_spmd(
        nc, in_maps, core_ids=list(range(NCORES)), **spmd_kwargs
    )
    outs = []
    for b in range(B):
        acc = res.results[4 * b]["out"].astype(np.float32)
        for g in range(1, 4):
            acc = acc + res.results[4 * b + g]["out"].astype(np.float32)
        outs.append(acc * OUT_SCALE)
    return np.stack(outs, axis=0), res


def kernel(**inputs):
    out, _ = _run(inputs)
    return out


# revision 15
# speedup vs baseline: 1.4312x; 1.0091x over previous
"""GQA attention layer (QKV proj + RoPE + softmax attention + out proj) on 8
Trainium2 NeuronCores.

Sharding: core c = (batch b = c//4, head-group g = c%4): 4 q heads + 1 kv
head per core, w_o row-parallel partial output in bf16; the host upcasts,
sums the 4 partials per batch and divides by the fp8 scaling factor (512).

Design highlights vs the original baseline (234 us -> 165 us):
- fp8 hi/lo DoubleRow matmuls for both projections: operands split into
  e4m3 hi + e5m2 lo (host-side for h/w_qkv/w_o, Pool-side for the attention
  output); each K=256 block runs as 3 DR matmuls (hi*hi + hi*lo + lo*hi) at
  0.75x the bf16 PE cost with ~bf16 accuracy. scores/attnV stay bf16
  (single-fp8 would blow the 2e-2 error budget).
- SCALE folded into the exp activation so q and k share one compact rope
  table; exp runs on Activation in [128, 1024] tiles; 1-2 tiles per head
  (~18% of keys) offload to DVE via a Schraudolph bitcast exp
  (int16(A*s + B) reinterpreted as bf16), weighted toward rounds where the
  PE has no filler work.
- softmax normalize: pacc copied off PSUM immediately (the psA bank recycles
  in one op), reciprocal on a partition-0 staged denominator (PSUM-sourced
  or partition-offset reciprocal inputs return garbage on HW), Pool
  partition_broadcast replaces the baseline's fp32 PE broadcast matmuls.
- schedule: two staggered attention-head generators with a filler queue
  (projections at 2x pump rate, V transposes, outproj tiles at 1/4 rate to
  cover whole q-tiles); junk warm-up matmuls bridge the initial DMA wait and
  the final norm chain so the PE p-state never drops mid-kernel; DMA order
  tuned so each k/v tile lands just before the attention wavefront needs it.
"""

import collections

import numpy as np
import ml_dtypes

B, S, HID = 2, 2048, 1024
NH, NKV, D = 16, 4, 64
SCALE = float(D ** -0.5)
NCORES = 8
TT = 512          # token tile
NTT = S // TT     # 4
KC = S // 128     # 16 key chunks
NCP = KC // 2     # 8 chunk pairs
OUT_SCALE = 1.0 / 512.0

_BF16 = ml_dtypes.bfloat16
_E4 = ml_dtypes.float8_e4m3
_E5 = ml_dtypes.float8_e5m2

_nc_cache = None


def _build_bass():
    import concourse.bass as bass
    import concourse.mybir as mybir
    import concourse.tile as tile
    from concourse import bacc
    from concourse.masks import make_identity

    BF = mybir.dt.bfloat16
    F32 = mybir.dt.float32
    I16 = mybir.dt.int16
    E4 = mybir.dt.float8e4
    E5 = mybir.dt.float8e5
    AF = mybir.ActivationFunctionType
    MULT = mybir.AluOpType.mult
    ADD = mybir.AluOpType.add
    SUB = mybir.AluOpType.subtract
    DR = mybir.MatmulPerfMode.DoubleRow
    # Schraudolph exp for offloaded tiles: bitcast(int16(s*A + B)) ~ exp(s)
    SCH_A = 184.66496280558537 * SCALE   # 128/ln2, scores carry 1/SCALE
    SCH_B = 16256.0 - 5.75 + 0.5         # bias center + truncation fix

    nc = bacc.Bacc()
    # h split hi/lo, packed [p, j, i, t]: h feature 256j + 128i + p
    hhi = nc.dram_tensor("hhi", (128, 4, 2, S), E4, kind="ExternalInput")
    hlo = nc.dram_tensor("hlo", (128, 4, 2, S), E5, kind="ExternalInput")
    # wqk split hi/lo, packed [p, rc, j, i, m]
    whi = nc.dram_tensor("whi", (128, 3, 4, 2, 128), E4, kind="ExternalInput")
    wlo = nc.dram_tensor("wlo", (128, 3, 4, 2, 128), E5, kind="ExternalInput")
    wohi = nc.dram_tensor("wohi", (128, 2, HID), E4, kind="ExternalInput")
    wolo = nc.dram_tensor("wolo", (128, 2, HID), E5, kind="ExternalInput")
    # shared q/k rope tables (/16); SCALE is applied by the exp activation
    cosd = nc.dram_tensor("cosd", (64, S), BF, kind="ExternalInput")
    sind = nc.dram_tensor("sind", (64, S), BF, kind="ExternalInput")
    out = nc.dram_tensor("out", (S, HID), BF, kind="ExternalOutput")

    with tile.TileContext(nc) as tc:
        with (
            tc.tile_pool(name="persist", bufs=1) as pp,
            tc.tile_pool(name="pbfp", bufs=3) as pbfp,
            tc.tile_pool(name="rope", bufs=3) as rp,
            tc.tile_pool(name="exps", bufs=6) as ep,
            tc.tile_pool(name="norm", bufs=4) as np_,
            tc.tile_pool(name="outsb", bufs=4) as op_,
        ):
            # ---- persistent SBUF tiles + input loads, kv-first order
            hhi_sb = pp.tile([128, 4, 2, S], E4, tag="hhi_sb")
            hlo_sb = pp.tile([128, 4, 2, S], E5, tag="hlo_sb")
            whi_sb = pp.tile([128, 3, 4, 2, 128], E4, tag="whi_sb")
            wlo_sb = pp.tile([128, 3, 4, 2, 128], E5, tag="wlo_sb")
            wohi_sb = pp.tile([128, 2, HID], E4, tag="wohi_sb")
            wolo_sb = pp.tile([128, 2, HID], E5, tag="wolo_sb")
            # [128, S]: rows 0:64 DMA'd, rows 64:128 duplicated on-device
            cos_sb = pp.tile([128, S], BF, tag="cos_sb")
            sin_sb = pp.tile([128, S], BF, tag="sin_sb")

            def h_slice(t0, t1):
                for hd, hs in ((hhi, hhi_sb), (hlo, hlo_sb)):
                    nc.sync.dma_start(hs[:, :, :, t0:t1], hd[:, :, :, t0:t1])

            nc.sync.dma_start(whi_sb[:, 2], whi[:, 2])
            nc.sync.dma_start(hhi_sb[:, 0:2, :, 0:TT], hhi[:, 0:2, :, 0:TT])
            nc.sync.dma_start(hhi_sb[:, 2:4, :, 0:TT], hhi[:, 2:4, :, 0:TT])
            nc.sync.dma_start(whi_sb[:, 0:2], whi[:, 0:2])
            nc.sync.dma_start(wlo_sb[:, 2], wlo[:, 2])
            nc.sync.dma_start(hlo_sb[:, :, :, 0:TT], hlo[:, :, :, 0:TT])
            nc.sync.dma_start(wlo_sb[:, 0:2], wlo[:, 0:2])
            for tt in range(NTT):
                tts_ = bass.ts(tt, TT)
                if tt > 0:
                    h_slice(tt * TT, (tt + 1) * TT)
                nc.sync.dma_start(cos_sb[0:64, tts_], cosd[:, tts_])
                nc.sync.dma_start(sin_sb[0:64, tts_], sind[:, tts_])
                # Pool duplicates the tables onto partitions 64:128
                nc.gpsimd.tensor_copy(cos_sb[64:128, tts_],
                                      cos_sb[0:64, tts_])
                nc.gpsimd.tensor_copy(sin_sb[64:128, tts_],
                                      sin_sb[0:64, tts_])
            nc.sync.dma_start(wohi_sb[:], wohi[:])
            nc.sync.dma_start(wolo_sb[:], wolo[:])

            ident = pp.tile([64, 64], BF, tag="ident")
            make_identity(nc, ident[:])
            warm = pp.tile([1, 8], F32, tag="warm")
            nc.any.memset(warm[:], 0.0)
            nc.scalar.activation(warm[:], warm[:], AF.Exp)
            # keep the tensor engine busy on junk matmuls while the first
            # h/w DMAs land, so the p-state ramp completes before real work
            wa = pp.tile([128, TT], BF, tag="wa")
            nc.gpsimd.memset(wa[:], 0.5)

            qrot = [pp.tile([128, S], BF, tag=f"qrot{p}", name=f"qrot{p}")
                    for p in range(2)]
            k2 = pp.tile([128, S], BF, tag="k2")
            vT = pp.tile([64, S], BF, tag="vT")
            vaug = pp.tile([128, KC, 65], BF, tag="vaug")
            nc.any.memset(vaug[:], 1.0 / 32.0)
            # anorm = 32*attn: bf16 full + fp8 hi/lo for the DR outproj,
            # packed [p, oc(=pair), t]
            anorm = pp.tile([128, 2, S], BF, tag="anorm")
            ahi = pp.tile([128, 2, S], E4, tag="ahi")
            alo = pp.tile([128, 2, S], E5, tag="alo")

            with (
                tc.tile_pool(name="psP", bufs=2, space="PSUM") as psP,
                tc.tile_pool(name="psS", bufs=2, space="PSUM") as psS,
                tc.tile_pool(name="psA", bufs=2, space="PSUM") as psA,
            ):

                def rope(pbf, dest, rows, tts, tag):
                    """Pool builds the 32-block-swapped copy; DVE runs
                    same-partition bf16 2x-mode multiply/add ops against the
                    compact shared [64, S] tables."""
                    sg = rp.tile([128, TT], BF, tag=f"sg{tag}")
                    for blk in range(rows // 32):
                        src = blk ^ 1
                        nc.gpsimd.tensor_copy(
                            sg[32 * blk: 32 * blk + 32, :],
                            pbf[32 * src: 32 * src + 32, :])
                    t1 = rp.tile([128, TT], BF, tag=f"t1{tag}")
                    rt = rp.tile([128, TT], BF, tag=f"rt{tag}")
                    nc.vector.tensor_tensor(
                        t1[0:rows, :], pbf[0:rows, :], cos_sb[0:rows, tts],
                        MULT)
                    nc.vector.tensor_tensor(
                        rt[0:rows, :], sg[0:rows, :], sin_sb[0:rows, tts],
                        MULT)
                    nc.vector.tensor_tensor(
                        dest, t1[0:rows, :], rt[0:rows, :], ADD)

                def proj(rc, tt, name, slot):
                    """hi/lo DoubleRow projection: 12 accumulating DR matmuls
                    (4 K=256 blocks x {hi*hi, hi*lo, lo*hi}). slot borrows an
                    idle PSUM ring early in the prologue."""
                    tts = bass.ts(tt, TT)
                    pool, tag = slot
                    if tag == "sc":
                        ps = pool.tile([128, 2 * TT], F32, tag="sc",
                                       name=name)[:, 0:TT]
                    else:
                        ps = pool.tile([128, TT], F32, tag=tag, name=name)
                    steps = []
                    for j in range(4):
                        steps.append((whi_sb[:, rc, j], hhi_sb[:, j, :, tts]))
                    for j in range(4):
                        steps.append((whi_sb[:, rc, j], hlo_sb[:, j, :, tts]))
                        steps.append((wlo_sb[:, rc, j], hhi_sb[:, j, :, tts]))
                    for si, (w, x) in enumerate(steps):
                        nc.tensor.matmul(
                            ps[:], w, x,
                            start=(si == 0), stop=(si == len(steps) - 1),
                            perf_mode=DR)
                    return ps, tts

                def proj_kv(tt, slot=(psP, "proj"), act_copy=False):
                    ps, tts = proj(2, tt, f"projkv_{tt}", slot)
                    kbf = pbfp.tile([128, TT], BF, tag="pbf", name=f"kbf{tt}")
                    if act_copy:
                        nc.scalar.copy(kbf[0:64, :], ps[0:64, :])
                        nc.scalar.copy(vT[:, tts], ps[64:128, :])
                        nc.vector.tensor_scalar_mul(vT[:, tts], vT[:, tts],
                                                    1.0 / 16.0)
                    else:
                        nc.vector.tensor_copy(kbf[0:64, :], ps[0:64, :])
                        # v = ps/16
                        nc.vector.tensor_scalar_mul(vT[:, tts], ps[64:128, :],
                                                    1.0 / 16.0)
                    rope(kbf, k2[0:64, tts], 64, tts, "k")
                    nc.gpsimd.tensor_copy(k2[64:128, tts], k2[0:64, tts])

                def transp(tt, slot=(psP, "proj")):
                    pool, tag = slot
                    pt = pool.tile([128, 4, 64], BF, tag=tag,
                                   name=f"vt{tt}")
                    for ci in range(4):
                        c = 4 * tt + ci
                        nc.tensor.transpose(pt[:, ci, :],
                                            vT[:, bass.ts(c, 128)], ident[:])
                    nc.vector.tensor_copy(vaug[:, 4 * tt: 4 * tt + 4, 0:64],
                                          pt[:])

                def proj_q(rc, tt, slot=(psP, "proj"), act_copy=False):
                    ps, tts = proj(rc, tt, f"projq{rc}_{tt}", slot)
                    pbf = pbfp.tile([128, TT], BF, tag="pbf",
                                    name=f"qbf{rc}_{tt}")
                    nc.vector.tensor_copy(pbf[:], ps[:])
                    rope(pbf, qrot[rc][:, tts], 128, tts, "q")

                def outproj_tile(tch, ht, last=False):
                    tcs = bass.ts(tch, 128)
                    hts = bass.ts(ht, TT)
                    po = psP.tile([128, TT], F32, tag="proj",
                                  name=f"po{tch}_{ht}")
                    terms = [(ahi[:, :, tcs], wohi_sb[:, :, hts]),
                             (ahi[:, :, tcs], wolo_sb[:, :, hts]),
                             (alo[:, :, tcs], wohi_sb[:, :, hts])]
                    for si, (a, w) in enumerate(terms):
                        nc.tensor.matmul(
                            po[:], a, w,
                            start=(si == 0), stop=(si == len(terms) - 1),
                            perf_mode=DR)
                    ob = op_.tile([128, TT], BF, tag="ob")
                    if (tch + ht) % 2 == 0:
                        nc.scalar.copy(ob[:], po[:])
                    else:
                        nc.vector.tensor_copy(ob[:], po[:])
                    nc.sync.dma_start(out[tcs, hts], ob[:])

                # ---- filler queue
                filler = collections.deque()

                def pump(n=1):
                    for _ in range(n):
                        if not filler:
                            return
                        filler.popleft()()

                def attention_head(pair, h2, qt, offload=True,
                                   last_head=False):
                    qts = bass.ts(qt, TT)
                    qrows = slice(64 * h2, 64 * h2 + 64)
                    pacc = psA.tile([65, TT], F32, tag="att",
                                    name=f"att{pair}_{h2}_{qt}")
                    pending = collections.deque()

                    def drain_pending(keep):
                        while len(pending) > keep:
                            pex, pcp = pending.popleft()
                            for j in range(2):
                                c = 2 * pcp + j
                                nc.tensor.matmul(
                                    pacc[:], vaug[:, c, :],
                                    pex[:, 512 * j: 512 * j + 512],
                                    start=(c == 0), stop=(c == KC - 1))

                    for cp in range(NCP):
                        sc = psS.tile([128, 2 * TT], F32, tag="sc",
                                      name=f"sc{pair}_{h2}_{qt}_{cp}")
                        for j in range(2):
                            c = 2 * cp + j
                            nc.tensor.matmul(
                                sc[:, bass.ts(j, TT)],
                                k2[qrows, bass.ts(c, 128)],
                                qrot[pair][qrows, qts],
                                start=True, stop=True,
                                tile_position=(64 * h2, 0))
                        if cp == 3 and offload:
                            # offload this tile's exp to DVE (Schraudolph);
                            # the ~2% approx error on 1/8 of the keys is
                            # within budget and relieves the pacing engine
                            exi = ep.tile([128, 2 * TT], I16, tag="exps")
                            nc.vector.tensor_scalar(exi[:], sc[:],
                                                    SCH_A, SCH_B, MULT, ADD)
                            ex = exi[:].bitcast(BF)
                        else:
                            ext = ep.tile([128, 2 * TT], BF, tag="exp")
                            nc.scalar.activation(ext[:], sc[:], AF.Exp,
                                                 scale=SCALE)
                            ex = ext[:]
                        pending.append((ex, cp))
                        # attnV trails scores by two cpairs so the exp
                        # semaphore has always fired by the time the PE
                        # reaches the accumulation matmuls
                        drain_pending(2)
                        yield
                    drain_pending(0)
                    # copy pacc to SBUF bf16 immediately -> psA bank freed in
                    # one op; normalize runs SBUF-side in bf16 2x-mode ops
                    att = np_.tile([64, TT], F32, tag="att_sb")
                    nc.vector.tensor_copy(att[:], pacc[0:64, :])
                    # den staged to a partition-0 tile: reciprocal reading an
                    # SBUF slice at base partition 64 returns garbage on HW
                    den = np_.tile([1, TT], F32, tag="den")
                    nc.vector.tensor_copy(den[:], pacc[64:65, :])
                    rec = np_.tile([1, TT], F32, tag="rec")
                    nc.vector.reciprocal_approx_fast(rec[:], den[:])
                    bc = np_.tile([64, TT], F32, tag="bc")
                    nc.gpsimd.partition_broadcast(bc[:], rec[:])
                    (nc.vector if last_head else nc.gpsimd).tensor_tensor(
                        anorm[qrows, pair, qts], att[0:64, :], bc[:], MULT)
                    # fp8 hi/lo for the DR outproj (Pool; DVE for the last
                    # head to shorten the tail chain)
                    eng = nc.vector if last_head else nc.gpsimd
                    eng.tensor_copy(ahi[qrows, pair, qts],
                                    anorm[qrows, pair, qts])
                    eng.tensor_tensor(
                        alo[qrows, pair, qts],
                        anorm[qrows, pair, qts],
                        ahi[qrows, pair, qts],
                        SUB)

                # ---- master schedule: kv0/q00 up front with Activation-
                # assisted copies (exp idle), V-transpose 0 borrows the psA
                # ring; later h tiles are DMA-gated so they pump as filler.
                def warm_mms(n, label):
                    for wi in range(n):
                        wps = psA.tile([128, TT], F32, tag="att",
                                       name=f"warm{label}_{wi}")
                        nc.tensor.matmul(wps[:, 0:256], wa[:, 0:128],
                                         wa[:, 0:256], start=True, stop=True)

                warm_mms(16, "a")
                proj_kv(0, slot=(psP, "proj"), act_copy=True)
                proj_q(0, 0, slot=(psP, "proj"), act_copy=True)
                transp(0, slot=(psA, "att"))
                proj_kv(1)
                filler.append(lambda: proj_kv(2))
                filler.append(lambda: transp(1))
                filler.append(lambda: proj_q(1, 0, slot=(psA, "att")))
                filler.append(lambda: proj_kv(3))
                filler.append(lambda: transp(2))
                filler.append(lambda: transp(3))
                for tt in range(1, NTT):
                    for rc in range(2):
                        filler.append(
                            lambda rc=rc, tt=tt: proj_q(rc, tt))

                heads = [(pair, h2, qt)
                         for qt in range(NTT)
                         for pair in range(2)
                         for h2 in range(2)]

                def head_done(i):
                    if i % 4 == 3:
                        qt = heads[i][2]
                        last = i == len(heads) - 1
                        for tch in range(4 * qt, 4 * qt + 4):
                            for ht in range(2):
                                filler.append(
                                    lambda tch=tch, ht=ht, last=last:
                                    outproj_tile(tch, ht, last=last))

                nxt = 0

                def start_next():
                    nonlocal nxt
                    if nxt >= len(heads):
                        return None
                    g = attention_head(*heads[nxt], offload=(nxt >= 4),
                                       last_head=(nxt == len(heads) - 1))
                    nxt += 1
                    return (nxt - 1, g)

                nproj_fill = len(filler)
                slots = [start_next(), None]
                stagger = 6
                step = 0
                while any(slots):
                    for si in range(2):
                        if slots[si] is None:
                            continue
                        i, g = slots[si]
                        try:
                            next(g)
                            # projection fillers drain at double rate (their
                            # PSUM->rope chains must stay ahead); outproj
                            # fillers at half rate so they cover the whole
                            # q-tile's rounds instead of bunching
                            if step < nproj_fill:
                                pump(2)
                            elif step % 2 == 0:
                                pump(1)
                            step += 1
                            if stagger is not None:
                                stagger -= 1
                                if stagger == 0:
                                    slots[1] = start_next()
                                    stagger = None
                        except StopIteration:
                            head_done(i)
                            slots[si] = start_next()
                # bridge the last norm chain with junk matmuls so the
                # final outproj tiles run at full PE clock
                filler.appendleft(lambda: warm_mms(14, "t"))
                while filler:
                    pump(1)
    nc.finalize()
    return nc


def _get_nc():
    global _nc_cache
    if _nc_cache is None:
        _nc_cache = _build_bass()
    return _nc_cache


def _hilo(x):
    hi = x.astype(_E4)
    lo = (x - hi.astype(np.float32)).astype(_E5)
    return hi, lo


def _shard_inputs(hidden_states, cos, sin, w_qkv, w_o):
    """Build per-core input maps. Core c = (b = c // 4, g = c % 4)."""
    cosT = cos.T.astype(np.float32)                                # [64, S]
    sinT = sin.T.astype(np.float32)
    sinmod = np.concatenate([-sinT[0:32], sinT[32:64]], axis=0)
    cosc = np.ascontiguousarray(cosT / 16.0).astype(_BF16)
    sinc = np.ascontiguousarray(sinmod / 16.0).astype(_BF16)

    # h packed [p, j, i, t]: feature 256j + 128i + p
    hsplit = []
    for b in range(B):
        ht = hidden_states[b].T.astype(np.float32)                 # [1024, S]
        hp = np.ascontiguousarray(
            ht.reshape(4, 2, 128, S).transpose(2, 0, 1, 3))        # [128,4,2,S]
        hsplit.append(_hilo(hp))
    in_maps = []
    for c in range(NCORES):
        b, g = divmod(c, 4)
        q_rows = w_qkv[256 * g: 256 * g + 256]
        k_rows = w_qkv[1024 + 64 * g: 1024 + 64 * g + 64]
        v_rows = w_qkv[1280 + 64 * g: 1280 + 64 * g + 64]
        wqk = np.concatenate([q_rows, k_rows, v_rows], axis=0)     # [384, 1024]
        # x16 into fp8 range; [p, rc, j, i, m] with h = 256j+128i+p
        wqkT = (wqk.T * 16.0).astype(np.float32)                   # [1024, 384]
        wpk = np.ascontiguousarray(
            wqkT.reshape(4, 2, 128, 3, 128).transpose(2, 3, 0, 1, 4))
        whi_a, wlo_a = _hilo(wpk)
        woTf = (w_o[:, 256 * g: 256 * g + 256].T * 16.0).astype(np.float32)
        wo_pk = np.ascontiguousarray(
            woTf.reshape(2, 128, HID).transpose(1, 0, 2))          # [128,2,HID]
        wohi_a, wolo_a = _hilo(wo_pk)
        in_maps.append(
            {
                "hhi": hsplit[b][0],
                "hlo": hsplit[b][1],
                "whi": whi_a,
                "wlo": wlo_a,
                "wohi": wohi_a,
                "wolo": wolo_a,
                "cosd": cosc,
                "sind": sinc,
            }
        )
    return in_maps


def _run(inputs, **spmd_kwargs):
    from concourse.bass_utils import run_bass_kernel_spmd

    nc = _get_nc()
    in_maps = _shard_inputs(**inputs)
    res = run_bass_kernel_spmd(
        nc, in_maps, core_ids=list(range(NCORES)), **spmd_kwargs
    )
    outs = []
    for b in range(B):
        acc = res.results[4 * b]["out"].astype(np.float32)
        for g in range(1, 4):
            acc = acc + res.results[4 * b + g]["out"].astype(np.float32)
        outs.append(acc * OUT_SCALE)
    return np.stack(outs, axis=0), res


def kernel(**inputs):
    out, _ = _run(inputs)
    return out


# revision 17
# speedup vs baseline: 1.4319x; 1.0005x over previous
"""GQA attention layer (QKV proj + RoPE + softmax attention + out proj) on 8
Trainium2 NeuronCores.

Sharding: core c = (batch b = c//4, head-group g = c%4): 4 q heads + 1 kv
head per core, w_o row-parallel partial output in bf16; the host upcasts,
sums the 4 partials per batch and divides by the fp8 scaling factor (512).

Design highlights vs the original baseline (234 us -> 164 us):
- fp8 hi/lo DoubleRow matmuls for both projections: operands split into
  e4m3 hi + e5m2 lo (host-side for h/w_qkv/w_o, Pool-side for the attention
  output); each K=256 block runs as 3 DR matmuls (hi*hi + hi*lo + lo*hi) at
  0.75x the bf16 PE cost with ~bf16 accuracy. scores/attnV stay bf16
  (single-fp8 would blow the 2e-2 error budget).
- SCALE folded into the exp activation so q and k share one compact rope
  table; exp runs on Activation in [128, 1024] tiles; 1-2 tiles per head
  (~18% of keys) offload to DVE via a Schraudolph bitcast exp
  (int16(A*s + B) reinterpreted as bf16), weighted toward rounds where the
  PE has no filler work.
- softmax normalize: pacc copied off PSUM immediately (the psA bank recycles
  in one op), reciprocal on a partition-0 staged denominator (PSUM-sourced
  or partition-offset reciprocal inputs return garbage on HW), Pool
  partition_broadcast + Pool multiply replace the baseline's fp32 PE
  broadcast matmuls (everything SBUF-side is Pool-legal).
- schedule: two staggered attention-head generators with a filler queue
  (projections at 2x pump rate, V transposes, outproj tiles at 1/4 rate to
  cover whole q-tiles); junk warm-up matmuls bridge the initial DMA wait and
  the final norm chain so the PE p-state never drops mid-kernel; DMA order
  tuned so each k/v tile lands just before the attention wavefront needs it.
"""

import collections

import numpy as np
import ml_dtypes

B, S, HID = 2, 2048, 1024
NH, NKV, D = 16, 4, 64
SCALE = float(D ** -0.5)
NCORES = 8
TT = 512          # token tile
NTT = S // TT     # 4
KC = S // 128     # 16 key chunks
NCP = KC // 2     # 8 chunk pairs
OUT_SCALE = 1.0 / 512.0

_BF16 = ml_dtypes.bfloat16
_E4 = ml_dtypes.float8_e4m3
_E5 = ml_dtypes.float8_e5m2

_nc_cache = None


def _build_bass():
    import concourse.bass as bass
    import concourse.mybir as mybir
    import concourse.tile as tile
    from concourse import bacc
    from concourse.masks import make_identity

    BF = mybir.dt.bfloat16
    F32 = mybir.dt.float32
    I16 = mybir.dt.int16
    E4 = mybir.dt.float8e4
    E5 = mybir.dt.float8e5
    AF = mybir.ActivationFunctionType
    MULT = mybir.AluOpType.mult
    ADD = mybir.AluOpType.add
    SUB = mybir.AluOpType.subtract
    DR = mybir.MatmulPerfMode.DoubleRow
    # Schraudolph exp for offloaded tiles: bitcast(int16(s*A + B)) ~ exp(s)
    SCH_A = 184.66496280558537 * SCALE   # 128/ln2, scores carry 1/SCALE
    SCH_B = 16256.0 - 5.75 + 0.5         # bias center + truncation fix

    nc = bacc.Bacc()
    # h split hi/lo, packed [p, j, i, t]: h feature 256j + 128i + p
    hhi = nc.dram_tensor("hhi", (128, 4, 2, S), E4, kind="ExternalInput")
    hlo = nc.dram_tensor("hlo", (128, 4, 2, S), E5, kind="ExternalInput")
    # wqk split hi/lo, packed [p, rc, j, i, m]
    whi = nc.dram_tensor("whi", (128, 3, 4, 2, 128), E4, kind="ExternalInput")
    wlo = nc.dram_tensor("wlo", (128, 3, 4, 2, 128), E5, kind="ExternalInput")
    wohi = nc.dram_tensor("wohi", (128, 2, HID), E4, kind="ExternalInput")
    wolo = nc.dram_tensor("wolo", (128, 2, HID), E5, kind="ExternalInput")
    # shared q/k rope tables (/16); SCALE is applied by the exp activation
    cosd = nc.dram_tensor("cosd", (64, S), BF, kind="ExternalInput")
    sind = nc.dram_tensor("sind", (64, S), BF, kind="ExternalInput")
    out = nc.dram_tensor("out", (S, HID), BF, kind="ExternalOutput")

    with tile.TileContext(nc) as tc:
        with (
            tc.tile_pool(name="persist", bufs=1) as pp,
            tc.tile_pool(name="pbfp", bufs=3) as pbfp,
            tc.tile_pool(name="rope", bufs=3) as rp,
            tc.tile_pool(name="exps", bufs=6) as ep,
            tc.tile_pool(name="norm", bufs=4) as np_,
            tc.tile_pool(name="outsb", bufs=4) as op_,
        ):
            # ---- persistent SBUF tiles + input loads, kv-first order
            hhi_sb = pp.tile([128, 4, 2, S], E4, tag="hhi_sb")
            hlo_sb = pp.tile([128, 4, 2, S], E5, tag="hlo_sb")
            whi_sb = pp.tile([128, 3, 4, 2, 128], E4, tag="whi_sb")
            wlo_sb = pp.tile([128, 3, 4, 2, 128], E5, tag="wlo_sb")
            wohi_sb = pp.tile([128, 2, HID], E4, tag="wohi_sb")
            wolo_sb = pp.tile([128, 2, HID], E5, tag="wolo_sb")
            # [128, S]: rows 0:64 DMA'd, rows 64:128 duplicated on-device
            cos_sb = pp.tile([128, S], BF, tag="cos_sb")
            sin_sb = pp.tile([128, S], BF, tag="sin_sb")

            def h_slice(t0, t1):
                for hd, hs in ((hhi, hhi_sb), (hlo, hlo_sb)):
                    nc.sync.dma_start(hs[:, :, :, t0:t1], hd[:, :, :, t0:t1])

            nc.sync.dma_start(whi_sb[:, 2], whi[:, 2])
            nc.sync.dma_start(hhi_sb[:, 0:2, :, 0:TT], hhi[:, 0:2, :, 0:TT])
            nc.sync.dma_start(hhi_sb[:, 2:4, :, 0:TT], hhi[:, 2:4, :, 0:TT])
            nc.sync.dma_start(wlo_sb[:, 2], wlo[:, 2])
            nc.sync.dma_start(hlo_sb[:, :, :, 0:TT], hlo[:, :, :, 0:TT])
            nc.sync.dma_start(whi_sb[:, 0:2], whi[:, 0:2])
            nc.sync.dma_start(wlo_sb[:, 0:2], wlo[:, 0:2])
            for tt in range(NTT):
                tts_ = bass.ts(tt, TT)
                if tt > 0:
                    h_slice(tt * TT, (tt + 1) * TT)
                nc.sync.dma_start(cos_sb[0:64, tts_], cosd[:, tts_])
                nc.sync.dma_start(sin_sb[0:64, tts_], sind[:, tts_])
                # Pool duplicates the tables onto partitions 64:128
                nc.gpsimd.tensor_copy(cos_sb[64:128, tts_],
                                      cos_sb[0:64, tts_])
                nc.gpsimd.tensor_copy(sin_sb[64:128, tts_],
                                      sin_sb[0:64, tts_])
            nc.sync.dma_start(wohi_sb[:], wohi[:])
            nc.sync.dma_start(wolo_sb[:], wolo[:])

            ident = pp.tile([64, 64], BF, tag="ident")
            make_identity(nc, ident[:])
            warm = pp.tile([1, 8], F32, tag="warm")
            nc.any.memset(warm[:], 0.0)
            nc.scalar.activation(warm[:], warm[:], AF.Exp)
            # keep the tensor engine busy on junk matmuls while the first
            # h/w DMAs land, so the p-state ramp completes before real work
            wa = pp.tile([128, TT], BF, tag="wa")
            nc.gpsimd.memset(wa[:], 0.5)

            qrot = [pp.tile([128, S], BF, tag=f"qrot{p}", name=f"qrot{p}")
                    for p in range(2)]
            k2 = pp.tile([128, S], BF, tag="k2")
            vT = pp.tile([64, S], BF, tag="vT")
            vaug = pp.tile([128, KC, 65], BF, tag="vaug")
            nc.any.memset(vaug[:], 1.0 / 32.0)
            # anorm = 32*attn: bf16 full + fp8 hi/lo for the DR outproj,
            # packed [p, oc(=pair), t]
            anorm = pp.tile([128, 2, S], BF, tag="anorm")
            ahi = pp.tile([128, 2, S], E4, tag="ahi")
            alo = pp.tile([128, 2, S], E5, tag="alo")

            with (
                tc.tile_pool(name="psP", bufs=2, space="PSUM") as psP,
                tc.tile_pool(name="psS", bufs=2, space="PSUM") as psS,
                tc.tile_pool(name="psA", bufs=2, space="PSUM") as psA,
            ):

                def rope(pbf, dest, rows, tts, tag):
                    """Pool builds the 32-block-swapped copy; DVE runs
                    same-partition bf16 2x-mode multiply/add ops against the
                    compact shared [64, S] tables."""
                    sg = rp.tile([128, TT], BF, tag=f"sg{tag}")
                    for blk in range(rows // 32):
                        src = blk ^ 1
                        nc.gpsimd.tensor_copy(
                            sg[32 * blk: 32 * blk + 32, :],
                            pbf[32 * src: 32 * src + 32, :])
                    t1 = rp.tile([128, TT], BF, tag=f"t1{tag}")
                    rt = rp.tile([128, TT], BF, tag=f"rt{tag}")
                    nc.vector.tensor_tensor(
                        t1[0:rows, :], pbf[0:rows, :], cos_sb[0:rows, tts],
                        MULT)
                    nc.vector.tensor_tensor(
                        rt[0:rows, :], sg[0:rows, :], sin_sb[0:rows, tts],
                        MULT)
                    nc.vector.tensor_tensor(
                        dest, t1[0:rows, :], rt[0:rows, :], ADD)

                def proj(rc, tt, name, slot):
                    """hi/lo DoubleRow projection: 12 accumulating DR matmuls
                    (4 K=256 blocks x {hi*hi, hi*lo, lo*hi}). slot borrows an
                    idle PSUM ring early in the prologue."""
                    tts = bass.ts(tt, TT)
                    pool, tag = slot
                    if tag == "sc":
                        ps = pool.tile([128, 2 * TT], F32, tag="sc",
                                       name=name)[:, 0:TT]
                    else:
                        ps = pool.tile([128, TT], F32, tag=tag, name=name)
                    steps = []
                    for j in range(4):
                        steps.append((whi_sb[:, rc, j], hhi_sb[:, j, :, tts]))
                    for j in range(4):
                        steps.append((whi_sb[:, rc, j], hlo_sb[:, j, :, tts]))
                        steps.append((wlo_sb[:, rc, j], hhi_sb[:, j, :, tts]))
                    for si, (w, x) in enumerate(steps):
                        nc.tensor.matmul(
                            ps[:], w, x,
                            start=(si == 0), stop=(si == len(steps) - 1),
                            perf_mode=DR)
                    return ps, tts

                def proj_kv(tt, slot=(psP, "proj"), act_copy=False):
                    ps, tts = proj(2, tt, f"projkv_{tt}", slot)
                    kbf = pbfp.tile([128, TT], BF, tag="pbf", name=f"kbf{tt}")
                    if act_copy:
                        nc.scalar.copy(kbf[0:64, :], ps[0:64, :])
                        nc.scalar.copy(vT[:, tts], ps[64:128, :])
                        nc.vector.tensor_scalar_mul(vT[:, tts], vT[:, tts],
                                                    1.0 / 16.0)
                    else:
                        nc.vector.tensor_copy(kbf[0:64, :], ps[0:64, :])
                        # v = ps/16
                        nc.vector.tensor_scalar_mul(vT[:, tts], ps[64:128, :],
                                                    1.0 / 16.0)
                    rope(kbf, k2[0:64, tts], 64, tts, "k")
                    nc.gpsimd.tensor_copy(k2[64:128, tts], k2[0:64, tts])

                def transp(tt, slot=(psP, "proj")):
                    pool, tag = slot
                    pt = pool.tile([128, 4, 64], BF, tag=tag,
                                   name=f"vt{tt}")
                    for ci in range(4):
                        c = 4 * tt + ci
                        nc.tensor.transpose(pt[:, ci, :],
                                            vT[:, bass.ts(c, 128)], ident[:])
                    nc.vector.tensor_copy(vaug[:, 4 * tt: 4 * tt + 4, 0:64],
                                          pt[:])

                def proj_q(rc, tt, slot=(psP, "proj"), act_copy=False):
                    ps, tts = proj(rc, tt, f"projq{rc}_{tt}", slot)
                    pbf = pbfp.tile([128, TT], BF, tag="pbf",
                                    name=f"qbf{rc}_{tt}")
                    nc.vector.tensor_copy(pbf[:], ps[:])
                    rope(pbf, qrot[rc][:, tts], 128, tts, "q")

                def outproj_tile(tch, ht, last=False):
                    tcs = bass.ts(tch, 128)
                    hts = bass.ts(ht, TT)
                    po = psP.tile([128, TT], F32, tag="proj",
                                  name=f"po{tch}_{ht}")
                    terms = [(ahi[:, :, tcs], wohi_sb[:, :, hts]),
                             (ahi[:, :, tcs], wolo_sb[:, :, hts]),
                             (alo[:, :, tcs], wohi_sb[:, :, hts])]
                    for si, (a, w) in enumerate(terms):
                        nc.tensor.matmul(
                            po[:], a, w,
                            start=(si == 0), stop=(si == len(terms) - 1),
                            perf_mode=DR)
                    ob = op_.tile([128, TT], BF, tag="ob")
                    if (tch + ht) % 2 == 0:
                        nc.scalar.copy(ob[:], po[:])
                    else:
                        nc.vector.tensor_copy(ob[:], po[:])
                    nc.sync.dma_start(out[tcs, hts], ob[:])

                # ---- filler queue
                filler = collections.deque()

                def pump(n=1):
                    for _ in range(n):
                        if not filler:
                            return
                        filler.popleft()()

                def attention_head(pair, h2, qt, offload=True,
                                   last_head=False):
                    qts = bass.ts(qt, TT)
                    qrows = slice(64 * h2, 64 * h2 + 64)
                    pacc = psA.tile([65, TT], F32, tag="att",
                                    name=f"att{pair}_{h2}_{qt}")
                    pending = collections.deque()

                    def drain_pending(keep):
                        while len(pending) > keep:
                            pex, pcp = pending.popleft()
                            for j in range(2):
                                c = 2 * pcp + j
                                nc.tensor.matmul(
                                    pacc[:], vaug[:, c, :],
                                    pex[:, 512 * j: 512 * j + 512],
                                    start=(c == 0), stop=(c == KC - 1))

                    for cp in range(NCP):
                        sc = psS.tile([128, 2 * TT], F32, tag="sc",
                                      name=f"sc{pair}_{h2}_{qt}_{cp}")
                        for j in range(2):
                            c = 2 * cp + j
                            nc.tensor.matmul(
                                sc[:, bass.ts(j, TT)],
                                k2[qrows, bass.ts(c, 128)],
                                qrot[pair][qrows, qts],
                                start=True, stop=True,
                                tile_position=(64 * h2, 0))
                        if cp == 3 and offload:
                            # offload this tile's exp to DVE (Schraudolph);
                            # the ~2% approx error on 1/8 of the keys is
                            # within budget and relieves the pacing engine
                            exi = ep.tile([128, 2 * TT], I16, tag="exps")
                            nc.vector.tensor_scalar(exi[:], sc[:],
                                                    SCH_A, SCH_B, MULT, ADD)
                            ex = exi[:].bitcast(BF)
                        else:
                            ext = ep.tile([128, 2 * TT], BF, tag="exp")
                            nc.scalar.activation(ext[:], sc[:], AF.Exp,
                                                 scale=SCALE)
                            ex = ext[:]
                        pending.append((ex, cp))
                        # attnV trails scores by two cpairs so the exp
                        # semaphore has always fired by the time the PE
                        # reaches the accumulation matmuls
                        drain_pending(2)
                        yield
                    drain_pending(0)
                    # copy pacc to SBUF bf16 immediately -> psA bank freed in
                    # one op; normalize runs SBUF-side in bf16 2x-mode ops
                    att = np_.tile([64, TT], F32, tag="att_sb")
                    nc.vector.tensor_copy(att[:], pacc[0:64, :])
                    # den staged to a partition-0 tile: reciprocal reading an
                    # SBUF slice at base partition 64 returns garbage on HW
                    den = np_.tile([1, TT], F32, tag="den")
                    nc.vector.tensor_copy(den[:], pacc[64:65, :])
                    rec = np_.tile([1, TT], F32, tag="rec")
                    nc.vector.reciprocal_approx_fast(rec[:], den[:])
                    bc = np_.tile([64, TT], F32, tag="bc")
                    nc.gpsimd.partition_broadcast(bc[:], rec[:])
                    (nc.vector if last_head else nc.gpsimd).tensor_tensor(
                        anorm[qrows, pair, qts], att[0:64, :], bc[:], MULT)
                    # fp8 hi/lo for the DR outproj (Pool; DVE for the last
                    # head to shorten the tail chain)
                    eng = nc.vector if last_head else nc.gpsimd
                    eng.tensor_copy(ahi[qrows, pair, qts],
                                    anorm[qrows, pair, qts])
                    eng.tensor_tensor(
                        alo[qrows, pair, qts],
                        anorm[qrows, pair, qts],
                        ahi[qrows, pair, qts],
                        SUB)

                # ---- master schedule: kv0/q00 up front with Activation-
                # assisted copies (exp idle), V-transpose 0 borrows the psA
                # ring; later h tiles are DMA-gated so they pump as filler.
                def warm_mms(n, label):
                    for wi in range(n):
                        wps = psA.tile([128, TT], F32, tag="att",
                                       name=f"warm{label}_{wi}")
                        nc.tensor.matmul(wps[:, 0:256], wa[:, 0:128],
                                         wa[:, 0:256], start=True, stop=True)

                warm_mms(16, "a")
                proj_kv(0, slot=(psP, "proj"), act_copy=True)
                proj_q(0, 0, slot=(psP, "proj"), act_copy=True)
                transp(0, slot=(psA, "att"))
                proj_kv(1)
                filler.append(lambda: proj_kv(2))
                filler.append(lambda: transp(1))
                filler.append(lambda: proj_q(1, 0, slot=(psA, "att")))
                filler.append(lambda: proj_kv(3))
                filler.append(lambda: transp(2))
                filler.append(lambda: transp(3))
                for tt in range(1, NTT):
                    for rc in range(2):
                        filler.append(
                            lambda rc=rc, tt=tt: proj_q(rc, tt))

                heads = [(pair, h2, qt)
                         for qt in range(NTT)
                         for pair in range(2)
                         for h2 in range(2)]

                def head_done(i):
                    if i % 4 == 3:
                        qt = heads[i][2]
                        last = i == len(heads) - 1
                        for tch in range(4 * qt, 4 * qt + 4):
                            for ht in range(2):
                                filler.append(
                                    lambda tch=tch, ht=ht, last=last:
                                    outproj_tile(tch, ht, last=last))

                nxt = 0

                def start_next():
                    nonlocal nxt
                    if nxt >= len(heads):
                        return None
                    g = attention_head(*heads[nxt], offload=(nxt >= 4),
                                       last_head=(nxt == len(heads) - 1))
                    nxt += 1
                    return (nxt - 1, g)

                nproj_fill = len(filler)
                slots = [start_next(), None]
                stagger = 6
                step = 0
                while any(slots):
                    for si in range(2):
                        if slots[si] is None:
                            continue
                        i, g = slots[si]
                        try:
                            next(g)
                            # projection fillers drain at double rate (their
                            # PSUM->rope chains must stay ahead); outproj
                            # fillers at half rate so they cover the whole
                            # q-tile's rounds instead of bunching
                            if step < nproj_fill:
                                pump(2)
                            elif step % 2 == 0:
                                pump(1)
                            step += 1
                            if stagger is not None:
                                stagger -= 1
                                if stagger == 0:
                                    slots[1] = start_next()
                                    stagger = None
                        except StopIteration:
                            head_done(i)
                            slots[si] = start_next()
                # bridge the last norm chain with junk matmuls so the
                # final outproj tiles run at full PE clock
                filler.appendleft(lambda: warm_mms(14, "t"))
                while filler:
                    pump(1)
    nc.finalize()
    return nc


def _get_nc():
    global _nc_cache
    if _nc_cache is None:
        _nc_cache = _build_bass()
    return _nc_cache


def _hilo(x):
    hi = x.astype(_E4)
    lo = (x - hi.astype(np.float32)).astype(_E5)
    return hi, lo


def _shard_inputs(hidden_states, cos, sin, w_qkv, w_o):
    """Build per-core input maps. Core c = (b = c // 4, g = c % 4)."""
    cosT = cos.T.astype(np.float32)                                # [64, S]
    sinT = sin.T.astype(np.float32)
    sinmod = np.concatenate([-sinT[0:32], sinT[32:64]], axis=0)
    cosc = np.ascontiguousarray(cosT / 16.0).astype(_BF16)
    sinc = np.ascontiguousarray(sinmod / 16.0).astype(_BF16)

    # h packed [p, j, i, t]: feature 256j + 128i + p
    hsplit = []
    for b in range(B):
        ht = hidden_states[b].T.astype(np.float32)                 # [1024, S]
        hp = np.ascontiguousarray(
            ht.reshape(4, 2, 128, S).transpose(2, 0, 1, 3))        # [128,4,2,S]
        hsplit.append(_hilo(hp))
    in_maps = []
    for c in range(NCORES):
        b, g = divmod(c, 4)
        q_rows = w_qkv[256 * g: 256 * g + 256]
        k_rows = w_qkv[1024 + 64 * g: 1024 + 64 * g + 64]
        v_rows = w_qkv[1280 + 64 * g: 1280 + 64 * g + 64]
        wqk = np.concatenate([q_rows, k_rows, v_rows], axis=0)     # [384, 1024]
        # x16 into fp8 range; [p, rc, j, i, m] with h = 256j+128i+p
        wqkT = (wqk.T * 16.0).astype(np.float32)                   # [1024, 384]
        wpk = np.ascontiguousarray(
            wqkT.reshape(4, 2, 128, 3, 128).transpose(2, 3, 0, 1, 4))
        whi_a, wlo_a = _hilo(wpk)
        woTf = (w_o[:, 256 * g: 256 * g + 256].T * 16.0).astype(np.float32)
        wo_pk = np.ascontiguousarray(
            woTf.reshape(2, 128, HID).transpose(1, 0, 2))          # [128,2,HID]
        wohi_a, wolo_a = _hilo(wo_pk)
        in_maps.append(
            {
                "hhi": hsplit[b][0],
                "hlo": hsplit[b][1],
                "whi": whi_a,
                "wlo": wlo_a,
                "wohi": wohi_a,
                "wolo": wolo_a,
                "cosd": cosc,
                "sind": sinc,
            }
        )
    return in_maps


def _run(inputs, **spmd_kwargs):
    from concourse.bass_utils import run_bass_kernel_spmd

    nc = _get_nc()
    in_maps = _shard_inputs(**inputs)
    res = run_bass_kernel_spmd(
        nc, in_maps, core_ids=list(range(NCORES)), **spmd_kwargs
    )
    outs = []
    for b in range(B):
        acc = res.results[4 * b]["out"].astype(np.float32)
        for g in range(1, 4):
            acc = acc + res.results[4 * b + g]["out"].astype(np.float32)
        outs.append(acc * OUT_SCALE)
    return np.stack(outs, axis=0), res


def kernel(**inputs):
    out, _ = _run(inputs)
    return out
